# revision 1
# baseline (speedup 1.0000x reference)
"""Trainium2 Bass kernel for nn_Decoder (LSTM decoder with mean-context).

Reference computation (per batch row b):
  context = mean_s input_encoded[b, s, :]                  # [E=64]
  LSTM primed 12 steps on y_hists, then 5 gen steps on y_targs,
  pred = ffin_w @ [h; context] + ffin_b after steps 11..16  # 6 preds of F=11
  out[b] = stack(preds)                                     # [6, 11]

Sharding: pure data-parallel over batch across 8 cores (B=32768 -> 4096/core).

Per-core layout strategy:
  - "chunk" = 512 batch columns, processed feature-major ([units, batch]).
  - chunks are processed in PAIRS (1024 batch): all LSTM state tiles are
    [128, 512] with rows 0:64 = chunk j units, rows 64:128 = chunk j+1 units,
    so every elementwise op runs dense on all 128 partitions.
  - gate matmuls use block-diagonal stationary operands [128,128]/[24,128]
    (prebuilt on host) so one N=512 matmul computes a gate for both chunks.
  - biases enter via an extra ones-row in the moving operand (K+1 trick).
  - the y-step is selected by the stationary operand (selector tiles with
    fc_w_y.T at row-block t), so moving operands always start at partition 0.
  - context mean over S=128 is computed from batch-major x tiles
    [128, 4096] with chained tensor_adds, split between DVE and GPSIMD,
    then PE-transposed into feature-major CTX tiles.
"""

import sys

import numpy as np

if "/opt/trn_rl_repo" not in sys.path:
    sys.path.insert(0, "/opt/trn_rl_repo")

import concourse.bass as bass
import concourse.tile as tile
from concourse import bacc
from concourse import mybir
from concourse import bass_utils

F32 = mybir.dt.float32
F32R = mybir.dt.float32r
AF = mybir.ActivationFunctionType

B, S, E, H, T, F = 32768, 128, 64, 64, 12, 11
NCORES = 8
B_CORE = B // NCORES      # 4096
CHUNK = 512               # batch columns per chunk (one psum bank)
PAIR = 2 * CHUNK          # 1024
NSTEP = T + 5             # 17 cell steps
NPRED = 6

# every GP_MEAN_MOD-th mean b-tile goes to gpsimd (rest on DVE)
GP_MEAN_MOD = 2

WK_NCOL = 2432  # packed stationary-operand tensor width

# Use float32r (fast fp32 PE mode, 1 cycle/row vs 4) for LSTM/pred matmuls.
FP32R = True


def host_prep(fc_w, fc_b, ffin_w, ffin_b, w_ih, w_hh, b_ih, b_hh):
    """Build all derived stationary operands in numpy (fp32)."""
    f32 = np.float32
    fc_w = fc_w.astype(f32)
    ffin_w = ffin_w.astype(f32)
    w_ih = w_ih.astype(f32)
    w_hh = w_hh.astype(f32)
    bias = (b_ih + b_hh).astype(f32)          # [256]

    # gate row ranges in torch order (i, f, g, o); psum block order: i, f, o, g
    gr = {"i": (0, 64), "f": (64, 128), "g": (128, 192), "o": (192, 256)}
    order = ("i", "f", "o", "g")

    gh = np.zeros((4, 128, 128), f32)
    gy = np.zeros((4, 24, 128), f32)
    for k, g in enumerate(order):
        r0, r1 = gr[g]
        whT = w_hh[r0:r1, :].T                # [64, 64]
        gh[k, 0:64, 0:64] = whT
        gh[k, 64:128, 64:128] = whT
        wiT = w_ih[r0:r1, :].T                # [11, 64]
        bg = bias[r0:r1]                      # [64]
        gy[k, 0, 0:64] = bg
        gy[k, 1:12, 0:64] = wiT
        gy[k, 12, 64:128] = bg
        gy[k, 13:24, 64:128] = wiT

    yc = np.zeros((128, 24), f32)             # ctx part of y_tilde (block-diag)
    yc[0:64, 1:12] = fc_w[:, 0:64].T
    yc[64:128, 13:24] = fc_w[:, 0:64].T
    yb = np.zeros((1, 24), f32)               # ones + fc_b row
    yb[0, 0] = 1.0
    yb[0, 12] = 1.0
    yb[0, 1:12] = fc_b
    yb[0, 13:24] = fc_b

    w_y = fc_w[:, 64:75].T                    # [11, 11]
    yhsel = np.zeros((12, 6 * F, 24), f32)    # (t-in-group, half) selectors
    for t in range(6):
        for h in range(2):
            yhsel[2 * t + h, t * F : (t + 1) * F, 1 + 12 * h : 12 + 12 * h] = w_y
    ytsel = np.zeros((10, 5 * F, 24), f32)
    for t in range(5):
        for h in range(2):
            ytsel[2 * t + h, t * F : (t + 1) * F, 1 + 12 * h : 12 + 12 * h] = w_y

    ph = np.zeros((128, 64), f32)             # pred h-part: chunk0->cols 0:11, chunk1->32:43
    ph[0:64, 0:11] = ffin_w[:, 0:64].T
    ph[64:128, 32:43] = ffin_w[:, 0:64].T
    pc = np.zeros((128, 64), f32)             # pred ctx-part
    pc[0:64, 0:11] = ffin_w[:, 64:128].T
    pc[64:128, 32:43] = ffin_w[:, 64:128].T
    pb = np.zeros((1, 64), f32)
    pb[0, 0:11] = ffin_b
    pb[0, 32:43] = ffin_b

    # pack everything into one [128, WK_NCOL] tensor -> single DMA, single
    # wait semaphore for all stationary operands.
    pk = np.zeros((128, WK_NCOL), f32)
    pk[:, 0:128] = np.eye(128, dtype=f32)
    pk[0, 128:640] = 1.0                                   # ones row
    for k in range(4):
        pk[:, 640 + 128 * k : 768 + 128 * k] = gh[k]
        pk[0:24, 1152 + 128 * k : 1280 + 128 * k] = gy[k]
    pk[:, 1664:1688] = yc
    pk[0:1, 1688:1712] = yb
    for i in range(12):
        pk[0 : 6 * F, 1712 + 24 * i : 1736 + 24 * i] = yhsel[i]
    for i in range(10):
        pk[0 : 5 * F, 2000 + 24 * i : 2024 + 24 * i] = ytsel[i]
    pk[:, 2240:2304] = ph
    pk[:, 2304:2368] = pc
    pk[0:1, 2368:2432] = pb
    return {"wk_all": pk}


def build_program(b_core: int = B_CORE):
    assert b_core % PAIR == 0
    npairs = b_core // PAIR
    nc = bacc.Bacc("TRN2", debug=False)

    x_d = nc.dram_tensor("input_encoded", [b_core, S, E], F32, kind="ExternalInput").ap()
    yh_d = nc.dram_tensor("y_hists", [b_core, T, F], F32, kind="ExternalInput").ap()
    yt_d = nc.dram_tensor("y_targs", [b_core, 5, F], F32, kind="ExternalInput").ap()
    wk_d = nc.dram_tensor("wk_all", [128, WK_NCOL], F32R if FP32R else F32, kind="ExternalInput").ap()
    out_d = nc.dram_tensor("out", [b_core, NPRED, F], F32, kind="ExternalOutput").ap()

    x_flat = x_d.rearrange("b s e -> b (s e)")        # [b_core, 8192]
    yh_flat = yh_d.rearrange("b t f -> b (t f)")      # [b_core, 132]
    yt_flat = yt_d.rearrange("b t f -> b (t f)")      # [b_core, 55]
    out_flat = out_d.rearrange("b p f -> b (p f)")    # [b_core, 66]

    with tile.TileContext(nc) as tc:
        with (
            tc.tile_pool(name="consts", bufs=1) as consts,
            tc.tile_pool(name="xload", bufs=4) as xload,
            tc.tile_pool(name="macc", bufs=3) as macc,
            tc.tile_pool(name="ctxbm", bufs=3) as ctxbm,
            tc.tile_pool(name="yload", bufs=3) as yload,
            tc.tile_pool(name="chunkd", bufs=3) as chunkd,
            tc.tile_pool(name="paird", bufs=3) as paird,
            tc.tile_pool(name="steptmp", bufs=3) as steptmp,
            tc.tile_pool(name="outbm", bufs=3) as outbm,
            tc.tile_pool(name="pgifo", bufs=1, space="PSUM") as pgifo,
            tc.tile_pool(name="pgg", bufs=1, space="PSUM") as pgg,
            tc.tile_pool(name="pyt", bufs=3, space="PSUM") as pyt,
            tc.tile_pool(name="ptrans", bufs=1, space="PSUM") as ptrans,
        ):
            # ---------------- one-time setup: single DMA of packed stationaries
            wk = consts.tile([128, WK_NCOL], F32R if FP32R else F32)
            nc.sync.dma_start(out=wk, in_=wk_d)
            ident = wk[:, 0:128].bitcast(F32)
            ones = wk[0:1, 128:640]
            GH = [wk[:, 640 + 128 * k : 768 + 128 * k] for k in range(4)]
            GY = [wk[0:24, 1152 + 128 * k : 1280 + 128 * k] for k in range(4)]
            YC = wk[:, 1664:1688]
            YB = wk[0:1, 1688:1712]
            YH_SEL = [
                [wk[0 : 6 * F, 1712 + 24 * (2 * t + h) : 1736 + 24 * (2 * t + h)] for h in range(2)]
                for t in range(6)
            ]
            YT_SEL = [
                [wk[0 : 5 * F, 2000 + 24 * (2 * t + h) : 2024 + 24 * (2 * t + h)] for h in range(2)]
                for t in range(5)
            ]
            PH = wk[:, 2240:2304]
            PC = wk[:, 2304:2368]
            PB = wk[0:1, 2368:2432]

            # ---------------- main loop: pair-groups of 2, steps interleaved
            f32v = lambda ap: ap.bitcast(F32) if FP32R else ap
            RD = F32R if FP32R else F32
            YD = RD

            def alloc_load_state(p):
                st = {}
                st["CTX2"] = paird.tile([128, CHUNK], F32, tag="ctx2", name=f"CTX2_{p}")
                st["yhA"] = [chunkd.tile([6 * F, CHUNK], YD, tag=f"yhA{h}", name=f"yhA{h}_{p}") for h in range(2)]
                st["yhB"] = [chunkd.tile([6 * F, CHUNK], YD, tag=f"yhB{h}", name=f"yhB{h}_{p}") for h in range(2)]
                st["ytT"] = [chunkd.tile([5 * F, CHUNK], YD, tag=f"ytT{h}", name=f"ytT{h}_{p}") for h in range(2)]
                return st

            def emit_load_unit(p, st, half, bt):
                """DMA + mean + transposes for one 128-row b-tile of pair p."""
                pb0 = p * PAIR
                bti = p * 8 + half * 4 + bt
                r0 = pb0 + half * CHUNK + bt * 128
                cslice = slice(bt * 128, (bt + 1) * 128)
                rrow = slice(half * 64, half * 64 + 64)

                # context mean; 7/16 of b-tiles on DVE, 9/16 gpsimd
                eng = nc.vector if (bti % 16) in (0, 2, 5, 7, 10, 13) else nc.gpsimd
                acc = macc.tile([128, 2 * CHUNK], F32, tag="macc", name=f"macc_{bti}")
                for qx in range(4):
                    xt = xload.tile([128, S * E // 4], F32, tag="xt", name=f"xt{qx}_{bti}")
                    nc.sync.dma_start(
                        out=xt,
                        in_=x_flat[r0 : r0 + 128, qx * 2048 : (qx + 1) * 2048],
                    )
                    if qx == 0:
                        eng.tensor_add(acc, xt[:, 0:1024], xt[:, 1024:2048])
                    else:
                        eng.tensor_add(acc, acc, xt[:, 0:1024])
                        eng.tensor_add(acc, acc, xt[:, 1024:2048])
                accT = macc.tile([128, CHUNK], F32, tag="accT")
                eng.tensor_add(accT, acc[:, 0:512], acc[:, 512:1024])
                f4 = macc.tile([128, 256], F32, tag="f4")
                eng.tensor_add(f4, accT[:, 0:256], accT[:, 256:512])
                f2 = macc.tile([128, 128], F32, tag="f2")
                eng.tensor_add(f2, f4[:, 0:128], f4[:, 128:256])
                cbm = ctxbm.tile([128, E], F32, tag="cbm")
                eng.tensor_add(cbm, f2[:, 0:64], f2[:, 64:128])
                ptc = ptrans.tile([128, 128], F32, tag="ptr", name=f"ptc_{bti}")
                nc.tensor.transpose(ptc[:E, :], cbm, ident)
                nc.scalar.activation(
                    st["CTX2"][rrow, cslice], ptc[0:64, 0:128], AF.Copy, scale=1.0 / S
                )

                # y_hists transpose: [128, 132] -> two [66, 128]
                yl = yload.tile([128, T * F], F32, tag="yl")
                nc.scalar.dma_start(out=yl, in_=yh_flat[r0 : r0 + 128, :])
                pth = ptrans.tile([128, 128], F32, tag="ptr", name=f"pth_{bti}")
                nc.tensor.transpose(pth[: 6 * F, :], yl[:, 0 : 6 * F], ident)
                nc.scalar.copy(st["yhA"][half][:, cslice], pth[: 6 * F, :])
                pth2 = ptrans.tile([128, 128], F32, tag="ptr", name=f"pth2_{bti}")
                nc.tensor.transpose(pth2[: 6 * F, :], yl[:, 6 * F : 12 * F], ident)
                nc.scalar.copy(st["yhB"][half][:, cslice], pth2[: 6 * F, :])

                # y_targs transpose: [128, 55] -> [55, 128]
                ytl = yload.tile([128, 5 * F], F32, tag="ytl")
                nc.scalar.dma_start(out=ytl, in_=yt_flat[r0 : r0 + 128, :])
                ptt = ptrans.tile([128, 128], F32, tag="ptr", name=f"ptt_{bti}")
                nc.tensor.transpose(ptt[: 5 * F, :], ytl, ident)
                nc.scalar.copy(st["ytT"][half][:, cslice], ptt[: 5 * F, :])

            def emit_ctx_terms(p, st):
                """Step-invariant context terms, exact fp32 (once per pair)."""
                st["outTa"] = [chunkd.tile([128, CHUNK], F32, tag=f"outTa{h}", name=f"outTa{h}_{p}") for h in range(2)]
                st["outTb"] = [chunkd.tile([64, CHUNK], F32, tag=f"outTb{h}", name=f"outTb{h}_{p}") for h in range(2)]
                for h in range(2):
                    nc.gpsimd.memset(st["outTa"][h], 0.0)
                    nc.gpsimd.memset(st["outTb"][h], 0.0)
                YcP = pyt.tile([24, CHUNK], F32, tag="ypred", name=f"YcP_{p}")
                nc.tensor.matmul(YcP, f32v(YB), f32v(ones), start=True, stop=False)
                nc.tensor.matmul(YcP, f32v(YC), st["CTX2"], start=False, stop=True)
                st["ytcS"] = paird.tile([24, CHUNK], F32, tag="ytcs", name=f"ytcS_{p}")
                nc.scalar.copy(st["ytcS"], YcP)
                PcP = pyt.tile([64, CHUNK], F32, tag="ypred", name=f"PcP_{p}")
                nc.tensor.matmul(PcP, f32v(PB), f32v(ones), start=True, stop=False)
                nc.tensor.matmul(PcP, f32v(PC), st["CTX2"], start=False, stop=True)
                st["pctxS"] = paird.tile([64, CHUNK], F32, tag="pctxs", name=f"pctxS_{p}")
                nc.scalar.copy(st["pctxS"], PcP)
                st["C2"] = paird.tile([128, CHUNK], F32, tag="c2", name=f"C2_{p}")
                nc.vector.memset(st["C2"], 0.0)
                st["H2"] = paird.tile([128, CHUNK], RD, tag="h2", name=f"H2_{p}")
                nc.vector.memset(st["H2"].bitcast(F32) if FP32R else st["H2"], 0.0)

            def emit_step_front(p, t, st):
                if t < 6:
                    ysrc, ysel = st["yhA"], YH_SEL[t]
                elif t < 12:
                    ysrc, ysel = st["yhB"], YH_SEL[t - 6]
                else:
                    ysrc, ysel = st["ytT"], YT_SEL[t - 12]

                # y-part of y_tilde (fast dtype); + ytc (exact) on DVE
                Y2 = pyt.tile([24, CHUNK], F32, tag="ypred", name=f"Y2_{p}_{t}")
                nc.tensor.matmul(Y2, ysel[0], ysrc[0], start=True, stop=False)
                nc.tensor.matmul(Y2, ysel[1], ysrc[1], start=False, stop=True)
                Ys2 = steptmp.tile([24, CHUNK], RD, tag="ys2", name=f"Ys2_{p}_{t}")
                nc.vector.tensor_add(Ys2, Y2, st["ytcS"])

                # gates: IFO psum = i | f | o (one sigmoid) ; G psum = g (tanh)
                IFOp = pgifo.tile([128, 3 * CHUNK], F32, tag="gifo", name=f"IFO_{p}_{t}")
                Gp = pgg.tile([128, CHUNK], F32, tag="gg", name=f"G_{p}_{t}")
                for gi, dst in ((0, IFOp[:, 0:CHUNK]), (1, IFOp[:, CHUNK : 2 * CHUNK]),
                                (2, IFOp[:, 2 * CHUNK : 3 * CHUNK]), (3, Gp)):
                    nc.tensor.matmul(dst, GY[gi], Ys2, start=True, stop=False)
                    nc.tensor.matmul(dst, GH[gi], st["H2"], start=False, stop=True)

                SIGs = steptmp.tile([128, 3 * CHUNK], F32, tag="sigs", name=f"SIGs_{p}_{t}")
                nc.scalar.activation(SIGs, IFOp, AF.Sigmoid)
                TGs = steptmp.tile([128, CHUNK], F32, tag="tgs", name=f"TGs_{p}_{t}")
                nc.scalar.activation(TGs, Gp, AF.Tanh)
                st["_f"] = (SIGs, TGs)

            def emit_step_back(p, t, st):
                SIGs, TGs = st.pop("_f")
                SOs = SIGs[:, 2 * CHUNK : 3 * CHUNK]
                # c = f*c + i*tanh(g);  h = o * tanh(c)
                C2, H2 = st["C2"], st["H2"]
                TMP = steptmp.tile([128, CHUNK], F32, tag="tmp", name=f"TMP_{p}_{t}")
                nc.vector.tensor_mul(C2, SIGs[:, CHUNK : 2 * CHUNK], C2)
                nc.vector.tensor_mul(TMP, SIGs[:, 0:CHUNK], TGs)
                nc.vector.tensor_add(C2, C2, TMP)
                TCs = steptmp.tile([128, CHUNK], F32, tag="tcs", name=f"TCs_{p}_{t}")
                nc.scalar.activation(TCs, C2, AF.Tanh)
                nc.vector.tensor_mul(H2, SOs, TCs)

                # prediction after steps 11..16
                if t >= T - 1:
                    pidx = t - (T - 1)
                    P2 = pgg.tile([128, CHUNK], F32, tag="gg", name=f"P2_{p}_{t}")
                    nc.tensor.matmul(P2[0:64, :], PH, H2, start=True, stop=True)
                    pctxS = st["pctxS"]
                    if pidx < 4:
                        dsta, dstb = st["outTa"][0], st["outTa"][1]
                        ro = pidx * 32
                    else:
                        dsta, dstb = st["outTb"][0], st["outTb"][1]
                        ro = (pidx - 4) * 32
                    nc.vector.tensor_add(dsta[ro : ro + F, :], P2[0:F, :], pctxS[0:F, :])
                    nc.vector.tensor_add(dstb[ro : ro + F, :], P2[32 : 32 + F, :], pctxS[32 : 32 + F, :])

            def emit_out_a(p, st):
                # preds 0..3 (outTa) are final after step 14 - overlap with steps 15/16
                pb0 = p * PAIR
                for half in range(2):
                    for bt in range(CHUNK // 128):
                        r0 = pb0 + half * CHUNK + bt * 128
                        cslice = slice(bt * 128, (bt + 1) * 128)
                        pta = ptrans.tile([128, 128], F32, tag="ptr", name=f"pta_{p}_{half}_{bt}")
                        nc.tensor.transpose(pta, st["outTa"][half][:, cslice], ident)
                        oba = outbm.tile([128, 128], F32, tag="oba")
                        nc.scalar.copy(oba, pta)
                        nc.scalar.dma_start(
                            out=out_flat[r0 : r0 + 128, 0 : 4 * F],
                            in_=oba.rearrange("p (b u) -> p b u", b=4)[:, :, 0:F],
                        )

            def emit_out_b(p, st):
                pb0 = p * PAIR
                for half in range(2):
                    for bt in range(CHUNK // 128):
                        r0 = pb0 + half * CHUNK + bt * 128
                        cslice = slice(bt * 128, (bt + 1) * 128)
                        ptb = ptrans.tile([128, 128], F32, tag="ptr", name=f"ptb_{p}_{half}_{bt}")
                        nc.tensor.transpose(ptb[:, 0:64], st["outTb"][half][:, cslice], ident[0:64, 0:64])
                        obb = outbm.tile([128, 64], F32, tag="obb")
                        nc.scalar.copy(obb, ptb[:, 0:64])
                        nc.scalar.dma_start(
                            out=out_flat[r0 : r0 + 128, 4 * F : 6 * F],
                            in_=obb.rearrange("p (b u) -> p b u", b=2)[:, :, 0:F],
                        )

            # staggered rounds: pair p starts its steps at STARTS[p]; its load
            # units are sprinkled to finish just before that.
            if npairs == 4:
                starts = [0, 12, 24, 36]
            else:
                starts = [9 * i for i in range(npairs)]
            states = {}
            # head: pair 0 loads fully
            states[0] = alloc_load_state(0)
            for half in range(2):
                for bt in range(CHUNK // 128):
                    emit_load_unit(0, states[0], half, bt)
            emit_ctx_terms(0, states[0])
            # load-unit schedule for pairs >= 1: unit u of pair p at round
            # floor(window_start + u * window_len / 8)
            unit_sched = {}
            for p in range(1, npairs):
                w0 = starts[p - 1] if p > 1 else 0
                w1 = max(starts[p] - 2, w0 + 1)
                for u in range(8):
                    r = w0 + u * (w1 - w0) // 8
                    unit_sched.setdefault(r, []).append((p, u))
            nrounds = starts[-1] + NSTEP
            for r in range(nrounds):
                live = [p for p in range(npairs) if 0 <= r - starts[p] < NSTEP]
                for p in live:
                    emit_step_front(p, r - starts[p], states[p])
                for p in live:
                    emit_step_back(p, r - starts[p], states[p])
                for (p, u) in unit_sched.get(r, []):
                    if p not in states:
                        states[p] = alloc_load_state(p)
                    emit_load_unit(p, states[p], u // 4, u % 4)
                    if u == 7:
                        emit_ctx_terms(p, states[p])
                for p in range(npairs):
                    if r == starts[p] + NSTEP - 3:
                        emit_out_a(p, states[p])
                    if r == starts[p] + NSTEP - 1:
                        emit_out_b(p, states[p])

    nc.compile()
    return nc


def shard_inputs(full, b_core):
    """Build per-core in_maps from full inputs (host-side)."""
    wk = host_prep(
        full["fc_w"], full["fc_b"], full["ffin_w"], full["ffin_b"],
        full["w_ih"], full["w_hh"], full["b_ih"], full["b_hh"],
    )
    in_maps = []
    for i in range(NCORES):
        sl = slice(i * b_core, (i + 1) * b_core)
        m = {
            "input_encoded": np.ascontiguousarray(full["input_encoded"][sl]),
            "y_hists": np.ascontiguousarray(full["y_hists"][sl]),
            "y_targs": np.ascontiguousarray(full["y_targs"][sl]),
        }
        m.update(wk)
        in_maps.append(m)
    return in_maps


def kernel(**inputs) -> np.ndarray:
    full = {k: np.asarray(v, dtype=np.float32) for k, v in inputs.items()}
    b_core = full["input_encoded"].shape[0] // NCORES
    nc = build_program(b_core)
    in_maps = shard_inputs(full, b_core)
    res = bass_utils.run_bass_kernel_spmd(nc, in_maps, core_ids=list(range(NCORES)))
    out = np.concatenate([res.results[i]["out"] for i in range(NCORES)], axis=0)
    return out.astype(np.float32)



# revision 14
# speedup vs baseline: 1.6077x; 1.6077x over previous
"""Trainium2 Bass kernel for nn_Decoder (LSTM decoder with mean-context).

Reference computation (per batch row b):
  context = mean_s input_encoded[b, s, :]                  # [E=64]
  LSTM primed 12 steps on y_hists, then 5 gen steps on y_targs,
  pred = ffin_w @ [h; context] + ffin_b after steps 11..16  # 6 preds of F=11
  out[b] = stack(preds)                                     # [6, 11]

Sharding: pure data-parallel over batch across 8 cores (B=32768 -> 4096/core).

v2.2 design (fp16 data plane, minimal op count):
  - input_encoded streams HBM->SBUF via gpsimd (SWDGE) cast-DMAs fp32->fp16.
  - s-mean = in-place fp16 binary tree on the x tile: the two wide levels on
    DVE (2x packed mode), the narrow levels on gpsimd; the Pool queue (which
    also issues the x DMAs) interleaves tree(u-BUFS_X) -> dma(u) so tile
    reuse never head-of-line blocks the x stream.
  - y_hists/y_targs are transposed on the host (pure layout prep) into
    [66|55, B_CORE] tensors; one cast-DMA each, no on-device transposes.
  - gates: one [128, 4cw] psum (i|f|o|g), ONE sigmoid over all four; the
    g-gate weights are pre-scaled by 2 so tanh(g) = 2*sigmoid(2g)-1 is a
    single 4x-mode DVE tensor_scalar.
  - predictions accumulate in a [66, 2cw] psum seeded with ffin_ctx+bias
    via matmuls; per-step pred matmuls land in row slices; output is a
    single [128, 66] transpose+copy+store per 128-batch block.
  - 8 batch groups of 512 (cw=256); per-round emission is software-
    pipelined: Y2/Ys2 of step t+1 are issued before gates of step t, so
    the in-order PE queue never stalls on the DVE y_tilde add.
"""

import sys

import numpy as np

if "/opt/trn_rl_repo" not in sys.path:
    sys.path.insert(0, "/opt/trn_rl_repo")

import concourse.bass as bass
import concourse.tile as tile
from concourse import bacc
from concourse import mybir
from concourse import bass_utils

F32 = mybir.dt.float32
F16 = mybir.dt.float16
AF = mybir.ActivationFunctionType
ALU = mybir.AluOpType

B, S, E, H, T, F = 32768, 128, 64, 64, 12, 11
NCORES = 8
B_CORE = B // NCORES      # 4096
NSTEP = T + 5             # 17 cell steps
NPRED = 6

CW = 256                       # chunk width (group batch = 2*CW = 512)
NGROUPS = B_CORE // (2 * CW)   # 8
NB_TILES = B_CORE // 128       # 32
UPG = NB_TILES // NGROUPS      # 4 units per group

BUFS_X = 6                # in-flight x tiles

WK_NCOL = 3232  # packed stationary-operand tensor width

# emission pacing estimates (ns) for the static schedule
EST_UNIT = 5900.0   # one b-tile cast-DMA on the DMA engines
EST_ROUND = 5000.0  # one LSTM step round


def host_prep(fc_w, fc_b, ffin_w, ffin_b, w_ih, w_hh, b_ih, b_hh):
    """Build all derived stationary operands in numpy (fp32; cast-loaded)."""
    f32 = np.float32
    fc_w = fc_w.astype(f32)
    ffin_w = ffin_w.astype(f32)
    w_ih = w_ih.astype(f32)
    w_hh = w_hh.astype(f32)
    bias = (b_ih + b_hh).astype(f32)          # [256]

    # gate row ranges in torch order (i, f, g, o); psum block order: i, f, o, g
    gr = {"i": (0, 64), "f": (64, 128), "g": (128, 192), "o": (192, 256)}
    order = ("i", "f", "o", "g")

    gh = np.zeros((4, 128, 128), f32)
    gy = np.zeros((4, 24, 128), f32)
    for k, g in enumerate(order):
        r0, r1 = gr[g]
        scale = 2.0 if g == "g" else 1.0      # tanh(g) = 2*sigmoid(2g) - 1
        whT = scale * w_hh[r0:r1, :].T        # [64, 64]
        gh[k, 0:64, 0:64] = whT
        gh[k, 64:128, 64:128] = whT
        wiT = scale * w_ih[r0:r1, :].T        # [11, 64]
        bg = scale * bias[r0:r1]              # [64]
        gy[k, 0, 0:64] = bg
        gy[k, 1:12, 0:64] = wiT
        gy[k, 12, 64:128] = bg
        gy[k, 13:24, 64:128] = wiT

    yc = np.zeros((128, 24), f32)             # ctx part of y_tilde (block-diag)
    yc[0:64, 1:12] = fc_w[:, 0:64].T
    yc[64:128, 13:24] = fc_w[:, 0:64].T
    yb = np.zeros((1, 24), f32)               # ones + fc_b row
    yb[0, 0] = 1.0
    yb[0, 12] = 1.0
    yb[0, 1:12] = fc_b
    yb[0, 13:24] = fc_b

    w_y = fc_w[:, 64:75].T                    # [11, 11]
    yhsel = np.zeros((12, 6 * F, 24), f32)    # (t-in-group, half) selectors
    for t in range(6):
        for h in range(2):
            yhsel[2 * t + h, t * F : (t + 1) * F, 1 + 12 * h : 12 + 12 * h] = w_y
    ytsel = np.zeros((10, 5 * F, 24), f32)
    for t in range(5):
        for h in range(2):
            ytsel[2 * t + h, t * F : (t + 1) * F, 1 + 12 * h : 12 + 12 * h] = w_y

    # prediction operands: accumulate into a [66, cw] psum per half; the
    # stationary places pred p at psum rows p*F (zeros elsewhere accumulate
    # harmlessly, keeping the matmul output base partition at 0).
    ph66 = np.zeros((6, 2, 128, 6 * F), f32)
    for p in range(6):
        for h in range(2):
            ph66[p, h, 64 * h : 64 * h + 64, p * F : (p + 1) * F] = ffin_w[:, 0:64].T
    pc66 = np.zeros((2, 128, 6 * F), f32)     # ctx-part, tiled over 6 preds
    for h in range(2):
        for p in range(6):
            pc66[h, 64 * h : 64 * h + 64, p * F : (p + 1) * F] = ffin_w[:, 64:128].T
    pb66 = np.tile(ffin_b.astype(f32), 6)[None, :]  # [1, 66]

    # pack everything into one [128, WK_NCOL] tensor -> single cast-DMA.
    pk = np.zeros((128, WK_NCOL), f32)
    pk[:, 0:128] = np.eye(128, dtype=f32)
    pk[0, 128:640] = 1.0                                   # ones row
    for k in range(4):
        pk[:, 640 + 128 * k : 768 + 128 * k] = gh[k]
        pk[0:24, 1152 + 128 * k : 1280 + 128 * k] = gy[k]
    pk[:, 1664:1688] = yc
    pk[0:1, 1688:1712] = yb
    for i in range(12):
        pk[0 : 6 * F, 1712 + 24 * i : 1736 + 24 * i] = yhsel[i]
    for i in range(10):
        pk[0 : 5 * F, 2000 + 24 * i : 2024 + 24 * i] = ytsel[i]
    for p in range(6):
        for h in range(2):
            pk[:, 2240 + 66 * (2 * p + h) : 2306 + 66 * (2 * p + h)] = ph66[p, h]
    pk[:, 3032:3098] = pc66[0]
    pk[:, 3098:3164] = pc66[1]
    pk[0:1, 3164:3230] = pb66
    return {"wk_all": pk}


def build_program(b_core: int = B_CORE):
    assert b_core == B_CORE
    nc = bacc.Bacc("TRN2", debug=False)

    x_d = nc.dram_tensor("input_encoded", [b_core, S, E], F32, kind="ExternalInput").ap()
    yhA_d = nc.dram_tensor("yhA_T", [6 * F, b_core], F32, kind="ExternalInput").ap()
    yhB_d = nc.dram_tensor("yhB_T", [6 * F, b_core], F32, kind="ExternalInput").ap()
    ytT_d = nc.dram_tensor("ytT_T", [5 * F, b_core], F32, kind="ExternalInput").ap()
    wk_d = nc.dram_tensor("wk_all", [128, WK_NCOL], F32, kind="ExternalInput").ap()
    out_d = nc.dram_tensor("out", [b_core, NPRED, F], F32, kind="ExternalOutput").ap()

    x_flat = x_d.rearrange("b s e -> b (s e)")        # [b_core, 8192]
    out_flat = out_d.rearrange("b p f -> b (p f)")    # [b_core, 66]

    with tile.TileContext(nc) as tc:
        with (
            tc.tile_pool(name="consts", bufs=1) as consts,
            tc.tile_pool(name="xload", bufs=BUFS_X) as xload,
            tc.tile_pool(name="ctxbm", bufs=3) as ctxbm,
            tc.tile_pool(name="grpd", bufs=8) as grpd,
            tc.tile_pool(name="steptmp", bufs=7) as steptmp,
            tc.tile_pool(name="outbm", bufs=3) as outbm,
            tc.tile_pool(name="pgifo", bufs=2, space="PSUM") as pgifo,
            tc.tile_pool(name="pyt", bufs=1, space="PSUM") as pyt,
            tc.tile_pool(name="ppred", bufs=2, space="PSUM") as ppred,
            tc.tile_pool(name="ptrans", bufs=1, space="PSUM") as ptrans,
        ):
            # ---------------- one-time setup: cast-load stationaries + y
            wk = consts.tile([128, WK_NCOL], F16)
            nc.gpsimd.dma_start(out=wk, in_=wk_d)
            ident = wk[:, 0:128]
            ones = wk[0:1, 128:640]
            GH = [wk[:, 640 + 128 * k : 768 + 128 * k] for k in range(4)]
            GY = [wk[0:24, 1152 + 128 * k : 1280 + 128 * k] for k in range(4)]
            YC = wk[:, 1664:1688]
            YB = wk[0:1, 1688:1712]
            YH_SEL = [
                [wk[0 : 6 * F, 1712 + 24 * (2 * t + h) : 1736 + 24 * (2 * t + h)] for h in range(2)]
                for t in range(6)
            ]
            YT_SEL = [
                [wk[0 : 5 * F, 2000 + 24 * (2 * t + h) : 2024 + 24 * (2 * t + h)] for h in range(2)]
                for t in range(5)
            ]
            PH66 = [
                [wk[:, 2240 + 66 * (2 * p + h) : 2306 + 66 * (2 * p + h)] for h in range(2)]
                for p in range(6)
            ]
            PC66 = [wk[:, 3032:3098], wk[:, 3098:3164]]
            PB66 = wk[0:1, 3164:3230]

            yhA_sb = consts.tile([6 * F, b_core], F16)
            nc.gpsimd.dma_start(out=yhA_sb, in_=yhA_d)
            yhB_sb = consts.tile([6 * F, b_core], F16)
            nc.gpsimd.dma_start(out=yhB_sb, in_=yhB_d)
            ytT_sb = consts.tile([5 * F, b_core], F16)
            nc.gpsimd.dma_start(out=ytT_sb, in_=ytT_d)

            # ---------------- per-unit (b-tile) streaming ops
            xt_tiles = {}

            def emit_x_dma(u):
                xt = xload.tile([128, S * E], F16, tag="xt", name=f"xt_{u}")
                xt_tiles[u] = xt
                nc.gpsimd.dma_start(out=xt, in_=x_flat[u * 128 : (u + 1) * 128, :])

            def emit_tree(u, states):
                """In-place mean tree + ctx transpose/copy for b-tile u."""
                g = u // UPG
                upg = u - g * UPG
                half, bt = divmod(upg, UPG // 2)
                cslice = slice(bt * 128, (bt + 1) * 128)
                rrow = slice(half * 64, half * 64 + 64)
                if g not in states:
                    states[g] = alloc_state(g)
                st = states[g]

                xt = xt_tiles.pop(u)
                w = S * E // 2
                while w >= 128:
                    eng = nc.vector if w >= 2048 else nc.gpsimd
                    eng.tensor_add(xt[:, 0:w], xt[:, 0:w], xt[:, w : 2 * w])
                    w //= 2
                cbm = ctxbm.tile([128, E], F16, tag="cbm")
                nc.gpsimd.tensor_add(cbm, xt[:, 0:64], xt[:, 64:128])
                ptc = ptrans.tile([128, 128], F16, tag="ptr", name=f"ptc_{u}")
                nc.tensor.transpose(ptc[:E, :], cbm, ident)
                nc.scalar.activation(
                    st["CTX2"][rrow, cslice], ptc[0:64, 0:128], AF.Copy, scale=1.0 / S
                )

            def alloc_state(g):
                st = {}
                st["CTX2"] = grpd.tile([128, CW], F16, tag="ctx2", name=f"CTX2_{g}")
                b0 = g * 2 * CW
                st["yhA"] = [yhA_sb[:, b0 + h * CW : b0 + (h + 1) * CW] for h in range(2)]
                st["yhB"] = [yhB_sb[:, b0 + h * CW : b0 + (h + 1) * CW] for h in range(2)]
                st["ytT"] = [ytT_sb[:, b0 + h * CW : b0 + (h + 1) * CW] for h in range(2)]
                st["Ys2"] = {}
                return st

            def emit_ctx_terms(g, st):
                """Step-invariant terms (once per group, data-ready)."""
                YcP = pyt.tile([24, CW], F32, tag="ypred", name=f"YcP_{g}")
                nc.tensor.matmul(YcP, YB, ones[0:1, 0:CW], start=True, stop=False)
                nc.tensor.matmul(YcP, YC, st["CTX2"], start=False, stop=True)
                st["ytcS"] = grpd.tile([24, CW], F16, tag="ytcs", name=f"ytcS_{g}")
                nc.scalar.copy(st["ytcS"], YcP)
                st["C2"] = grpd.tile([128, CW], F16, tag="c2", name=f"C2_{g}")
                st["H2"] = grpd.tile([128, CW], F16, tag="h2", name=f"H2_{g}")

            def emit_pred_seed(g, st):
                # pred psum [66, 2*CW] seeded with ctx part + bias; allocated
                # late (t=10) so only ~2 groups hold a PRED tile at once.
                PRED = ppred.tile([66, 2 * CW], F32, tag="pred", name=f"PRED_{g}")
                st["PRED"] = PRED
                # single full-width start=True: psum accumulation-start acts
                # at bank granularity, so per-half starts would clobber the
                # other half's seed
                nc.tensor.matmul(PRED, PB66, ones[0:1, 0 : 2 * CW], start=True,
                                 stop=False, skip_group_check=True)
                for h in range(2):
                    nc.tensor.matmul(PRED[:, h * CW : (h + 1) * CW], PC66[h],
                                     st["CTX2"], start=False, stop=False,
                                     skip_group_check=True)

            def emit_front_y(g, t, st):
                """y-part of y_tilde for step t (hoisted one round early)."""
                if t < 6:
                    ysrc, ysel = st["yhA"], YH_SEL[t]
                elif t < 12:
                    ysrc, ysel = st["yhB"], YH_SEL[t - 6]
                else:
                    ysrc, ysel = st["ytT"], YT_SEL[t - 12]
                Y2 = pyt.tile([24, CW], F32, tag="ypred", name=f"Y2_{g}_{t}")
                nc.tensor.matmul(Y2, ysel[0], ysrc[0], start=True, stop=False)
                nc.tensor.matmul(Y2, ysel[1], ysrc[1], start=False, stop=True)
                Ys2 = steptmp.tile([24, CW], F16, tag="ys2", name=f"Ys2_{g}_{t}")
                nc.vector.tensor_add(Ys2, Y2, st["ytcS"])
                st["Ys2"][t] = Ys2

            def emit_mid(g, t, st):
                """gates + single sigmoid for step t."""
                Ys2 = st["Ys2"].pop(t)
                IFOG = pgifo.tile([128, 4 * CW], F32, tag="gifo", name=f"IFOG_{g}_{t}")
                for gi in range(4):
                    dst = IFOG[:, gi * CW : (gi + 1) * CW]
                    if t == 0:  # h0 == 0: y-part only
                        nc.tensor.matmul(dst, GY[gi], Ys2, start=True, stop=True)
                    else:
                        nc.tensor.matmul(dst, GY[gi], Ys2, start=True, stop=False)
                        nc.tensor.matmul(dst, GH[gi], st["H2"], start=False, stop=True)
                SIG4 = steptmp.tile([128, 4 * CW], F16, tag="sig4", name=f"SIG4_{g}_{t}")
                nc.scalar.activation(SIG4, IFOG, AF.Sigmoid)
                st["_f"] = SIG4

            def emit_back(g, t, st):
                SIG4 = st.pop("_f")
                C2, H2 = st["C2"], st["H2"]
                # tanh(g) = 2*sigmoid(2g) - 1 (4x-mode tensor_scalar)
                TGs = steptmp.tile([128, CW], F16, tag="tgs", name=f"TGs_{g}_{t}")
                nc.vector.tensor_scalar(TGs, SIG4[:, 3 * CW : 4 * CW], 2.0, -1.0,
                                        ALU.mult, ALU.add)
                # c = f*c + i*tanh(g);  h = o * tanh(c)
                if t == 0:  # c0 == 0
                    nc.vector.tensor_mul(C2, SIG4[:, 0:CW], TGs)
                else:
                    TMP = steptmp.tile([128, CW], F16, tag="tmp", name=f"TMP_{g}_{t}")
                    nc.vector.tensor_mul(C2, SIG4[:, CW : 2 * CW], C2)
                    nc.vector.tensor_mul(TMP, SIG4[:, 0:CW], TGs)
                    nc.vector.tensor_add(C2, C2, TMP)
                TCs = steptmp.tile([128, CW], F16, tag="tcs", name=f"TCs_{g}_{t}")
                nc.scalar.activation(TCs, C2, AF.Tanh)
                nc.vector.tensor_mul(H2, SIG4[:, 2 * CW : 3 * CW], TCs)

                # prediction after steps 11..16 -> row slice of PRED psum
                if t >= T - 1:
                    p = t - (T - 1)
                    PRED = st["PRED"]
                    for h in range(2):
                        nc.tensor.matmul(
                            PRED[:, h * CW : (h + 1) * CW],
                            PH66[p][h], H2, start=False, stop=(p == NPRED - 1),
                            skip_group_check=True,
                        )

            def emit_out(g, st):
                PRED = st["PRED"]
                for h in range(2):
                    o66 = grpd.tile([66, CW], F16, tag="o66", name=f"o66_{g}_{h}")
                    nc.scalar.copy(o66, PRED[:, h * CW : (h + 1) * CW])
                    for bt in range(CW // 128):
                        r0 = (g * UPG + half_off(h, bt)) * 128
                        pto = ptrans.tile([128, 128], F16, tag="ptr",
                                          name=f"pto_{g}_{h}_{bt}")
                        nc.tensor.transpose(
                            pto[:, 0:66], o66[:, bt * 128 : (bt + 1) * 128],
                            ident[0:66, 0:66],
                        )
                        obm = outbm.tile([128, 66], F32, tag="obm")
                        nc.scalar.copy(obm, pto[:, 0:66])
                        nc.sync.dma_start(
                            out=out_flat[r0 : r0 + 128, :], in_=obm
                        )

            def half_off(h, bt):
                return h * (UPG // 2) + bt

            # ---------------- virtual-time list schedule
            # Every emission item gets an estimated feasible start time; we
            # emit in that order so each in-order engine queue sees work in
            # the sequence it actually becomes runnable.
            TREE_DELAY = 1500.0   # data-arrival -> tree emission
            CTX_LAT = 1200.0
            STEP_LAT = 5200.0     # per-step chain latency estimate

            arr = {u: (u + 1) * EST_UNIT + 500.0 for u in range(NB_TILES)}
            key_tree = {u: arr[u] + TREE_DELAY for u in range(NB_TILES)}
            key_dma = {}
            for u in range(NB_TILES):
                if u < BUFS_X:
                    key_dma[u] = float(u)
                else:
                    # pool-order invariant: dma(u) right after tree(u-BUFS_X)
                    key_dma[u] = key_tree[u - BUFS_X] + 1.0

            items = []
            seq = 0
            def add(key, kind, payload):
                nonlocal seq
                items.append((key, seq, kind, payload))
                seq += 1

            for u in range(NB_TILES):
                add(key_dma[u], "dma", u)
                add(key_tree[u], "tree", u)
            key_ctx = {}
            for g in range(NGROUPS):
                key_ctx[g] = key_tree[g * UPG + UPG - 1] + CTX_LAT
                add(key_ctx[g], "ctx", g)
                for t in range(NSTEP):
                    add(key_ctx[g] + 2000.0 + t * STEP_LAT, "step", (g, t))
                add(key_ctx[g] + 2000.0 + (NSTEP - 1) * STEP_LAT + 1.0,
                    "out", g)

            items.sort(key=lambda it: (it[0], it[1]))

            states = {}
            for key, _s, kind, payload in items:
                if kind == "dma":
                    emit_x_dma(payload)
                elif kind == "tree":
                    emit_tree(payload, states)
                elif kind == "ctx":
                    g = payload
                    emit_ctx_terms(g, states[g])
                    emit_front_y(g, 0, states[g])
                elif kind == "step":
                    g, t = payload
                    st = states[g]
                    if t + 1 < NSTEP:
                        emit_front_y(g, t + 1, st)
                    emit_mid(g, t, st)
                    if t == T - 2:
                        emit_pred_seed(g, st)
                    emit_back(g, t, st)
                elif kind == "out":
                    emit_out(payload, states[payload])

    nc.compile()
    return nc


def shard_inputs(full, b_core):
    """Build per-core in_maps from full inputs (host-side layout prep)."""
    wk = host_prep(
        full["fc_w"], full["fc_b"], full["ffin_w"], full["ffin_b"],
        full["w_ih"], full["w_hh"], full["b_ih"], full["b_hh"],
    )
    in_maps = []
    for i in range(NCORES):
        sl = slice(i * b_core, (i + 1) * b_core)
        yh = full["y_hists"][sl].astype(np.float32)      # [b_core, 12, 11]
        yt = full["y_targs"][sl].astype(np.float32)      # [b_core, 5, 11]
        m = {
            "input_encoded": np.ascontiguousarray(full["input_encoded"][sl]),
            "yhA_T": np.ascontiguousarray(yh[:, 0:6, :].reshape(b_core, 66).T),
            "yhB_T": np.ascontiguousarray(yh[:, 6:12, :].reshape(b_core, 66).T),
            "ytT_T": np.ascontiguousarray(yt.reshape(b_core, 55).T),
        }
        m.update(wk)
        in_maps.append(m)
    return in_maps


def kernel(**inputs) -> np.ndarray:
    full = {k: np.asarray(v, dtype=np.float32) for k, v in inputs.items()}
    b_core = full["input_encoded"].shape[0] // NCORES
    nc = build_program(b_core)
    in_maps = shard_inputs(full, b_core)
    res = bass_utils.run_bass_kernel_spmd(nc, in_maps, core_ids=list(range(NCORES)))
    out = np.concatenate([res.results[i]["out"] for i in range(NCORES)], axis=0)
    return out.astype(np.float32)


# revision 24
# speedup vs baseline: 1.6286x; 1.0130x over previous
"""Trainium2 Bass kernel for nn_Decoder (LSTM decoder with mean-context).

Reference computation (per batch row b):
  context = mean_s input_encoded[b, s, :]                  # [E=64]
  LSTM primed 12 steps on y_hists, then 5 gen steps on y_targs,
  pred = ffin_w @ [h; context] + ffin_b after steps 11..16  # 6 preds of F=11
  out[b] = stack(preds)                                     # [6, 11]

Sharding: pure data-parallel over batch across 8 cores (B=32768 -> 4096/core).

v2.2 design (fp16 data plane, minimal op count):
  - input_encoded streams HBM->SBUF via gpsimd (SWDGE) cast-DMAs fp32->fp16.
  - s-mean = in-place fp16 binary tree on the x tile: the two wide levels on
    DVE (2x packed mode), the narrow levels on gpsimd; the Pool queue (which
    also issues the x DMAs) interleaves tree(u-BUFS_X) -> dma(u) so tile
    reuse never head-of-line blocks the x stream.
  - y_hists/y_targs are transposed on the host (pure layout prep) into
    [66|55, B_CORE] tensors; one cast-DMA each, no on-device transposes.
  - gates: one [128, 4cw] psum (i|f|o|g), ONE sigmoid over all four; the
    g-gate weights are pre-scaled by 2 so tanh(g) = 2*sigmoid(2g)-1 is a
    single 4x-mode DVE tensor_scalar.
  - predictions accumulate in a [66, 2cw] psum seeded with ffin_ctx+bias
    via matmuls; per-step pred matmuls land in row slices; output is a
    single [128, 66] transpose+copy+store per 128-batch block.
  - 8 batch groups of 512 (cw=256); per-round emission is software-
    pipelined: Y2/Ys2 of step t+1 are issued before gates of step t, so
    the in-order PE queue never stalls on the DVE y_tilde add.
"""

import sys

import numpy as np

if "/opt/trn_rl_repo" not in sys.path:
    sys.path.insert(0, "/opt/trn_rl_repo")

import concourse.bass as bass
import concourse.tile as tile
from concourse import bacc
from concourse import mybir
from concourse import bass_utils

F32 = mybir.dt.float32
F16 = mybir.dt.float16
AF = mybir.ActivationFunctionType
ALU = mybir.AluOpType

B, S, E, H, T, F = 32768, 128, 64, 64, 12, 11
NCORES = 8
B_CORE = B // NCORES      # 4096
NSTEP = T + 5             # 17 cell steps
NPRED = 6

CW = 256                       # chunk width (group batch = 2*CW = 512)
NGROUPS = B_CORE // (2 * CW)   # 8
NB_TILES = B_CORE // 128       # 32
UPG = NB_TILES // NGROUPS      # 4 units per group

BUFS_X = 6                # in-flight x tiles

WK_NCOL = 3232  # packed stationary-operand tensor width

# emission pacing estimates (ns) for the static schedule
EST_UNIT = 5900.0   # one b-tile cast-DMA on the DMA engines
EST_ROUND = 5000.0  # one LSTM step round


def host_prep(fc_w, fc_b, ffin_w, ffin_b, w_ih, w_hh, b_ih, b_hh):
    """Build all derived stationary operands in numpy (fp32; cast-loaded)."""
    f32 = np.float32
    fc_w = fc_w.astype(f32)
    ffin_w = ffin_w.astype(f32)
    w_ih = w_ih.astype(f32)
    w_hh = w_hh.astype(f32)
    bias = (b_ih + b_hh).astype(f32)          # [256]

    # gate row ranges in torch order (i, f, g, o); psum block order: i, f, o, g
    gr = {"i": (0, 64), "f": (64, 128), "g": (128, 192), "o": (192, 256)}
    order = ("i", "f", "o", "g")

    gh = np.zeros((4, 128, 128), f32)
    gy = np.zeros((4, 24, 128), f32)
    for k, g in enumerate(order):
        r0, r1 = gr[g]
        scale = 2.0 if g == "g" else 1.0      # tanh(g) = 2*sigmoid(2g) - 1
        whT = scale * w_hh[r0:r1, :].T        # [64, 64]
        gh[k, 0:64, 0:64] = whT
        gh[k, 64:128, 64:128] = whT
        wiT = scale * w_ih[r0:r1, :].T        # [11, 64]
        bg = scale * bias[r0:r1]              # [64]
        gy[k, 0, 0:64] = bg
        gy[k, 1:12, 0:64] = wiT
        gy[k, 12, 64:128] = bg
        gy[k, 13:24, 64:128] = wiT

    yc = np.zeros((128, 24), f32)             # ctx part of y_tilde (block-diag)
    yc[0:64, 1:12] = fc_w[:, 0:64].T
    yc[64:128, 13:24] = fc_w[:, 0:64].T
    yb = np.zeros((1, 24), f32)               # ones + fc_b row
    yb[0, 0] = 1.0
    yb[0, 12] = 1.0
    yb[0, 1:12] = fc_b
    yb[0, 13:24] = fc_b

    w_y = fc_w[:, 64:75].T                    # [11, 11]
    yhsel = np.zeros((12, 6 * F, 24), f32)    # (t-in-group, half) selectors
    for t in range(6):
        for h in range(2):
            yhsel[2 * t + h, t * F : (t + 1) * F, 1 + 12 * h : 12 + 12 * h] = w_y
    ytsel = np.zeros((10, 5 * F, 24), f32)
    for t in range(5):
        for h in range(2):
            ytsel[2 * t + h, t * F : (t + 1) * F, 1 + 12 * h : 12 + 12 * h] = w_y

    # prediction operands: accumulate into a [66, cw] psum per half; the
    # stationary places pred p at psum rows p*F (zeros elsewhere accumulate
    # harmlessly, keeping the matmul output base partition at 0).
    ph66 = np.zeros((6, 2, 128, 6 * F), f32)
    for p in range(6):
        for h in range(2):
            ph66[p, h, 64 * h : 64 * h + 64, p * F : (p + 1) * F] = ffin_w[:, 0:64].T
    pc66 = np.zeros((2, 128, 6 * F), f32)     # ctx-part, tiled over 6 preds
    for h in range(2):
        for p in range(6):
            pc66[h, 64 * h : 64 * h + 64, p * F : (p + 1) * F] = ffin_w[:, 64:128].T
    pb66 = np.tile(ffin_b.astype(f32), 6)[None, :]  # [1, 66]

    # pack everything into one [128, WK_NCOL] tensor -> single cast-DMA.
    pk = np.zeros((128, WK_NCOL), f32)
    pk[:, 0:128] = np.eye(128, dtype=f32)
    pk[0, 128:640] = 1.0                                   # ones row
    for k in range(4):
        pk[:, 640 + 128 * k : 768 + 128 * k] = gh[k]
        pk[0:24, 1152 + 128 * k : 1280 + 128 * k] = gy[k]
    pk[:, 1664:1688] = yc
    pk[0:1, 1688:1712] = yb
    for i in range(12):
        pk[0 : 6 * F, 1712 + 24 * i : 1736 + 24 * i] = yhsel[i]
    for i in range(10):
        pk[0 : 5 * F, 2000 + 24 * i : 2024 + 24 * i] = ytsel[i]
    for p in range(6):
        for h in range(2):
            pk[:, 2240 + 66 * (2 * p + h) : 2306 + 66 * (2 * p + h)] = ph66[p, h]
    pk[:, 3032:3098] = pc66[0]
    pk[:, 3098:3164] = pc66[1]
    pk[0:1, 3164:3230] = pb66
    return {"wk_all": pk}


def build_program(b_core: int = B_CORE):
    assert b_core == B_CORE
    nc = bacc.Bacc("TRN2", debug=False)

    x_d = nc.dram_tensor("input_encoded", [b_core, S, E], F32, kind="ExternalInput").ap()
    yhA_d = nc.dram_tensor("yhA_T", [6 * F, b_core], F32, kind="ExternalInput").ap()
    yhB_d = nc.dram_tensor("yhB_T", [6 * F, b_core], F32, kind="ExternalInput").ap()
    ytT_d = nc.dram_tensor("ytT_T", [5 * F, b_core], F32, kind="ExternalInput").ap()
    wk_d = nc.dram_tensor("wk_all", [128, WK_NCOL], F32, kind="ExternalInput").ap()
    out_d = nc.dram_tensor("out", [b_core, NPRED, F], F32, kind="ExternalOutput").ap()

    x_flat = x_d.rearrange("b s e -> b (s e)")        # [b_core, 8192]
    out_flat = out_d.rearrange("b p f -> b (p f)")    # [b_core, 66]

    with tile.TileContext(nc) as tc:
        with (
            tc.tile_pool(name="consts", bufs=1) as consts,
            tc.tile_pool(name="xload", bufs=BUFS_X) as xload,
            tc.tile_pool(name="ctxbm", bufs=3) as ctxbm,
            tc.tile_pool(name="grpd", bufs=8) as grpd,
            tc.tile_pool(name="steptmp", bufs=7) as steptmp,
            tc.tile_pool(name="outbm", bufs=3) as outbm,
            tc.tile_pool(name="pgifo", bufs=2, space="PSUM") as pgifo,
            tc.tile_pool(name="pyt", bufs=1, space="PSUM") as pyt,
            tc.tile_pool(name="ppred", bufs=2, space="PSUM") as ppred,
            tc.tile_pool(name="ptrans", bufs=1, space="PSUM") as ptrans,
        ):
            # ---------------- one-time setup: cast-load stationaries + y
            wk = consts.tile([128, WK_NCOL], F16)
            nc.gpsimd.dma_start(out=wk, in_=wk_d)
            ident = wk[:, 0:128]
            ones = wk[0:1, 128:640]
            GH = [wk[:, 640 + 128 * k : 768 + 128 * k] for k in range(4)]
            GY = [wk[0:24, 1152 + 128 * k : 1280 + 128 * k] for k in range(4)]
            YC = wk[:, 1664:1688]
            YB = wk[0:1, 1688:1712]
            YH_SEL = [
                [wk[0 : 6 * F, 1712 + 24 * (2 * t + h) : 1736 + 24 * (2 * t + h)] for h in range(2)]
                for t in range(6)
            ]
            YT_SEL = [
                [wk[0 : 5 * F, 2000 + 24 * (2 * t + h) : 2024 + 24 * (2 * t + h)] for h in range(2)]
                for t in range(5)
            ]
            PH66 = [
                [wk[:, 2240 + 66 * (2 * p + h) : 2306 + 66 * (2 * p + h)] for h in range(2)]
                for p in range(6)
            ]
            PC66 = [wk[:, 3032:3098], wk[:, 3098:3164]]
            PB66 = wk[0:1, 3164:3230]

            yhA_sb = consts.tile([6 * F, b_core], F16)
            nc.gpsimd.dma_start(out=yhA_sb, in_=yhA_d)
            yhB_sb = consts.tile([6 * F, b_core], F16)
            nc.gpsimd.dma_start(out=yhB_sb, in_=yhB_d)
            ytT_sb = consts.tile([5 * F, b_core], F16)
            nc.gpsimd.dma_start(out=ytT_sb, in_=ytT_d)

            # ---------------- per-unit (b-tile) streaming ops
            xt_tiles = {}

            def emit_x_dma(u):
                xt = xload.tile([128, S * E], F16, tag="xt", name=f"xt_{u}")
                xt_tiles[u] = xt
                nc.gpsimd.dma_start(out=xt, in_=x_flat[u * 128 : (u + 1) * 128, :])

            def emit_tree(u, states):
                """In-place mean tree + ctx transpose/copy for b-tile u."""
                g = u // UPG
                upg = u - g * UPG
                half, bt = divmod(upg, UPG // 2)
                cslice = slice(bt * 128, (bt + 1) * 128)
                rrow = slice(half * 64, half * 64 + 64)
                if g not in states:
                    states[g] = alloc_state(g)
                st = states[g]

                xt = xt_tiles.pop(u)
                # level 1 into a fresh half-size tile so the 16KB x slot is
                # freed as soon as the first DVE add retires
                t2 = ctxbm.tile([128, S * E // 2], F16, tag="t2", name=f"t2_{u}")
                nc.vector.tensor_add(t2, xt[:, 0 : S * E // 2], xt[:, S * E // 2 :])
                w = S * E // 4
                while w >= 128:
                    eng = nc.vector if w >= 2048 else nc.gpsimd
                    eng.tensor_add(t2[:, 0:w], t2[:, 0:w], t2[:, w : 2 * w])
                    w //= 2
                cbm = ctxbm.tile([128, E], F16, tag="cbm")
                nc.gpsimd.tensor_add(cbm, t2[:, 0:64], t2[:, 64:128])
                ptc = ptrans.tile([128, 128], F16, tag="ptr", name=f"ptc_{u}")
                nc.tensor.transpose(ptc[:E, :], cbm, ident)
                nc.scalar.activation(
                    st["CTX2"][rrow, cslice], ptc[0:64, 0:128], AF.Copy, scale=1.0 / S
                )

            def alloc_state(g):
                st = {}
                st["CTX2"] = grpd.tile([128, CW], F16, tag="ctx2", name=f"CTX2_{g}")
                b0 = g * 2 * CW
                st["yhA"] = [yhA_sb[:, b0 + h * CW : b0 + (h + 1) * CW] for h in range(2)]
                st["yhB"] = [yhB_sb[:, b0 + h * CW : b0 + (h + 1) * CW] for h in range(2)]
                st["ytT"] = [ytT_sb[:, b0 + h * CW : b0 + (h + 1) * CW] for h in range(2)]
                st["Ys2"] = {}
                return st

            def emit_ctx_terms(g, st):
                """Step-invariant terms (once per group, data-ready)."""
                YcP = pyt.tile([24, CW], F32, tag="ypred", name=f"YcP_{g}")
                nc.tensor.matmul(YcP, YB, ones[0:1, 0:CW], start=True, stop=False)
                nc.tensor.matmul(YcP, YC, st["CTX2"], start=False, stop=True)
                st["ytcS"] = grpd.tile([24, CW], F16, tag="ytcs", name=f"ytcS_{g}")
                nc.scalar.copy(st["ytcS"], YcP)
                st["C2"] = grpd.tile([128, CW], F16, tag="c2", name=f"C2_{g}")
                st["H2"] = grpd.tile([128, CW], F16, tag="h2", name=f"H2_{g}")

            def emit_pred_seed(g, st):
                # pred psum [66, 2*CW] seeded with ctx part + bias; allocated
                # late (t=10) so only ~2 groups hold a PRED tile at once.
                PRED = ppred.tile([66, 2 * CW], F32, tag="pred", name=f"PRED_{g}")
                st["PRED"] = PRED
                # single full-width start=True: psum accumulation-start acts
                # at bank granularity, so per-half starts would clobber the
                # other half's seed
                nc.tensor.matmul(PRED, PB66, ones[0:1, 0 : 2 * CW], start=True,
                                 stop=False, skip_group_check=True)
                for h in range(2):
                    nc.tensor.matmul(PRED[:, h * CW : (h + 1) * CW], PC66[h],
                                     st["CTX2"], start=False, stop=False,
                                     skip_group_check=True)

            def emit_front_y(g, t, st):
                """y-part of y_tilde for step t (hoisted one round early)."""
                if t < 6:
                    ysrc, ysel = st["yhA"], YH_SEL[t]
                elif t < 12:
                    ysrc, ysel = st["yhB"], YH_SEL[t - 6]
                else:
                    ysrc, ysel = st["ytT"], YT_SEL[t - 12]
                Y2 = pyt.tile([24, CW], F32, tag="ypred", name=f"Y2_{g}_{t}")
                nc.tensor.matmul(Y2, ysel[0], ysrc[0], start=True, stop=False)
                nc.tensor.matmul(Y2, ysel[1], ysrc[1], start=False, stop=True)
                Ys2 = steptmp.tile([24, CW], F16, tag="ys2", name=f"Ys2_{g}_{t}")
                nc.vector.tensor_add(Ys2, Y2, st["ytcS"])
                st["Ys2"][t] = Ys2

            def emit_mid(g, t, st):
                """gates + single sigmoid for step t."""
                Ys2 = st["Ys2"].pop(t)
                IFOG = pgifo.tile([128, 4 * CW], F32, tag="gifo", name=f"IFOG_{g}_{t}")
                for gi in range(4):
                    dst = IFOG[:, gi * CW : (gi + 1) * CW]
                    if t == 0:  # h0 == 0: y-part only
                        nc.tensor.matmul(dst, GY[gi], Ys2, start=True, stop=True)
                    else:
                        nc.tensor.matmul(dst, GY[gi], Ys2, start=True, stop=False)
                        nc.tensor.matmul(dst, GH[gi], st["H2"], start=False, stop=True)
                SIG4 = steptmp.tile([128, 4 * CW], F16, tag="sig4", name=f"SIG4_{g}_{t}")
                nc.scalar.activation(SIG4, IFOG, AF.Sigmoid)
                st["_f"] = SIG4

            def emit_back(g, t, st):
                SIG4 = st.pop("_f")
                C2, H2 = st["C2"], st["H2"]
                # tanh(g) = 2*sigmoid(2g) - 1 (4x-mode tensor_scalar)
                TGs = steptmp.tile([128, CW], F16, tag="tgs", name=f"TGs_{g}_{t}")
                nc.vector.tensor_scalar(TGs, SIG4[:, 3 * CW : 4 * CW], 2.0, -1.0,
                                        ALU.mult, ALU.add)
                # c = f*c + i*tanh(g);  h = o * tanh(c)
                if t == 0:  # c0 == 0
                    nc.vector.tensor_mul(C2, SIG4[:, 0:CW], TGs)
                else:
                    TMP = steptmp.tile([128, CW], F16, tag="tmp", name=f"TMP_{g}_{t}")
                    nc.vector.tensor_mul(C2, SIG4[:, CW : 2 * CW], C2)
                    nc.vector.tensor_mul(TMP, SIG4[:, 0:CW], TGs)
                    nc.vector.tensor_add(C2, C2, TMP)
                TCs = steptmp.tile([128, CW], F16, tag="tcs", name=f"TCs_{g}_{t}")
                nc.scalar.activation(TCs, C2, AF.Tanh)
                nc.vector.tensor_mul(H2, SIG4[:, 2 * CW : 3 * CW], TCs)

                # prediction after steps 11..16 -> row slice of PRED psum
                if t >= T - 1:
                    p = t - (T - 1)
                    PRED = st["PRED"]
                    for h in range(2):
                        nc.tensor.matmul(
                            PRED[:, h * CW : (h + 1) * CW],
                            PH66[p][h], H2, start=False, stop=(p == NPRED - 1),
                            skip_group_check=True,
                        )

            def emit_out(g, st):
                PRED = st["PRED"]
                for h in range(2):
                    o66 = grpd.tile([66, CW], F16, tag="o66", name=f"o66_{g}_{h}")
                    nc.scalar.copy(o66, PRED[:, h * CW : (h + 1) * CW])
                    for bt in range(CW // 128):
                        r0 = (g * UPG + half_off(h, bt)) * 128
                        pto = ptrans.tile([128, 128], F16, tag="ptr",
                                          name=f"pto_{g}_{h}_{bt}")
                        nc.tensor.transpose(
                            pto[:, 0:66], o66[:, bt * 128 : (bt + 1) * 128],
                            ident[0:66, 0:66],
                        )
                        obm = outbm.tile([128, 66], F32, tag="obm")
                        nc.scalar.copy(obm, pto[:, 0:66])
                        nc.sync.dma_start(
                            out=out_flat[r0 : r0 + 128, :], in_=obm
                        )

            def half_off(h, bt):
                return h * (UPG // 2) + bt

            # ---------------- virtual-time list schedule
            # Every emission item gets an estimated feasible start time; we
            # emit in that order so each in-order engine queue sees work in
            # the sequence it actually becomes runnable.
            TREE_DELAY = 600.0   # data-arrival -> tree emission
            CTX_LAT = 1200.0
            STEP_LAT = 5200.0     # per-step chain latency estimate

            arr = {u: (u + 1) * EST_UNIT + 500.0 for u in range(NB_TILES)}
            key_tree = {u: arr[u] + TREE_DELAY for u in range(NB_TILES)}
            key_dma = {}
            for u in range(NB_TILES):
                if u < BUFS_X:
                    key_dma[u] = float(u)
                else:
                    # pool-order invariant: dma(u) right after tree(u-BUFS_X)
                    key_dma[u] = key_tree[u - BUFS_X] + 1.0

            items = []
            seq = 0
            def add(key, kind, payload):
                nonlocal seq
                items.append((key, seq, kind, payload))
                seq += 1

            for u in range(NB_TILES):
                add(key_dma[u], "dma", u)
                add(key_tree[u], "tree", u)
            key_ctx = {}
            for g in range(NGROUPS):
                key_ctx[g] = key_tree[g * UPG + UPG - 1] + CTX_LAT
                add(key_ctx[g], "ctx", g)
                for t in range(NSTEP):
                    add(key_ctx[g] + 2000.0 + t * STEP_LAT, "step", (g, t))
                add(key_ctx[g] + 2000.0 + (NSTEP - 1) * STEP_LAT + 1.0,
                    "out", g)

            items.sort(key=lambda it: (it[0], it[1]))

            states = {}
            for key, _s, kind, payload in items:
                if kind == "dma":
                    emit_x_dma(payload)
                elif kind == "tree":
                    emit_tree(payload, states)
                elif kind == "ctx":
                    g = payload
                    emit_ctx_terms(g, states[g])
                    emit_front_y(g, 0, states[g])
                elif kind == "step":
                    g, t = payload
                    st = states[g]
                    if t + 1 < NSTEP:
                        emit_front_y(g, t + 1, st)
                    emit_mid(g, t, st)
                    if t == T - 2:
                        emit_pred_seed(g, st)
                    emit_back(g, t, st)
                elif kind == "out":
                    emit_out(payload, states[payload])

    nc.compile()
    return nc


def shard_inputs(full, b_core):
    """Build per-core in_maps from full inputs (host-side layout prep)."""
    wk = host_prep(
        full["fc_w"], full["fc_b"], full["ffin_w"], full["ffin_b"],
        full["w_ih"], full["w_hh"], full["b_ih"], full["b_hh"],
    )
    in_maps = []
    for i in range(NCORES):
        sl = slice(i * b_core, (i + 1) * b_core)
        yh = full["y_hists"][sl].astype(np.float32)      # [b_core, 12, 11]
        yt = full["y_targs"][sl].astype(np.float32)      # [b_core, 5, 11]
        m = {
            "input_encoded": np.ascontiguousarray(full["input_encoded"][sl]),
            "yhA_T": np.ascontiguousarray(yh[:, 0:6, :].reshape(b_core, 66).T),
            "yhB_T": np.ascontiguousarray(yh[:, 6:12, :].reshape(b_core, 66).T),
            "ytT_T": np.ascontiguousarray(yt.reshape(b_core, 55).T),
        }
        m.update(wk)
        in_maps.append(m)
    return in_maps


def kernel(**inputs) -> np.ndarray:
    full = {k: np.asarray(v, dtype=np.float32) for k, v in inputs.items()}
    b_core = full["input_encoded"].shape[0] // NCORES
    nc = build_program(b_core)
    in_maps = shard_inputs(full, b_core)
    res = bass_utils.run_bass_kernel_spmd(nc, in_maps, core_ids=list(range(NCORES)))
    out = np.concatenate([res.results[i]["out"] for i in range(NCORES)], axis=0)
    return out.astype(np.float32)


# revision 25
# speedup vs baseline: 1.6385x; 1.0061x over previous
"""Trainium2 Bass kernel for nn_Decoder (LSTM decoder with mean-context).

Reference computation (per batch row b):
  context = mean_s input_encoded[b, s, :]                  # [E=64]
  LSTM primed 12 steps on y_hists, then 5 gen steps on y_targs,
  pred = ffin_w @ [h; context] + ffin_b after steps 11..16  # 6 preds of F=11
  out[b] = stack(preds)                                     # [6, 11]

Sharding: pure data-parallel over batch across 8 cores (B=32768 -> 4096/core).

v2.2 design (fp16 data plane, minimal op count):
  - input_encoded streams HBM->SBUF via gpsimd (SWDGE) cast-DMAs fp32->fp16.
  - s-mean = in-place fp16 binary tree on the x tile: the two wide levels on
    DVE (2x packed mode), the narrow levels on gpsimd; the Pool queue (which
    also issues the x DMAs) interleaves tree(u-BUFS_X) -> dma(u) so tile
    reuse never head-of-line blocks the x stream.
  - y_hists/y_targs are transposed on the host (pure layout prep) into
    [66|55, B_CORE] tensors; one cast-DMA each, no on-device transposes.
  - gates: one [128, 4cw] psum (i|f|o|g), ONE sigmoid over all four; the
    g-gate weights are pre-scaled by 2 so tanh(g) = 2*sigmoid(2g)-1 is a
    single 4x-mode DVE tensor_scalar.
  - predictions accumulate in a [66, 2cw] psum seeded with ffin_ctx+bias
    via matmuls; per-step pred matmuls land in row slices; output is a
    single [128, 66] transpose+copy+store per 128-batch block.
  - 8 batch groups of 512 (cw=256); per-round emission is software-
    pipelined: Y2/Ys2 of step t+1 are issued before gates of step t, so
    the in-order PE queue never stalls on the DVE y_tilde add.
"""

import sys

import numpy as np

if "/opt/trn_rl_repo" not in sys.path:
    sys.path.insert(0, "/opt/trn_rl_repo")

import concourse.bass as bass
import concourse.tile as tile
from concourse import bacc
from concourse import mybir
from concourse import bass_utils

F32 = mybir.dt.float32
F16 = mybir.dt.float16
AF = mybir.ActivationFunctionType
ALU = mybir.AluOpType

B, S, E, H, T, F = 32768, 128, 64, 64, 12, 11
NCORES = 8
B_CORE = B // NCORES      # 4096
NSTEP = T + 5             # 17 cell steps
NPRED = 6

CW = 256                       # chunk width (group batch = 2*CW = 512)
NGROUPS = B_CORE // (2 * CW)   # 8
NB_TILES = B_CORE // 128       # 32
UPG = NB_TILES // NGROUPS      # 4 units per group

BUFS_X = 6                # in-flight x tiles

WK_NCOL = 3232  # packed stationary-operand tensor width

# emission pacing estimates (ns) for the static schedule
EST_UNIT = 6700.0   # one b-tile cast-DMA on the DMA engines
EST_ROUND = 5000.0  # one LSTM step round


def host_prep(fc_w, fc_b, ffin_w, ffin_b, w_ih, w_hh, b_ih, b_hh):
    """Build all derived stationary operands in numpy (fp32; cast-loaded)."""
    f32 = np.float32
    fc_w = fc_w.astype(f32)
    ffin_w = ffin_w.astype(f32)
    w_ih = w_ih.astype(f32)
    w_hh = w_hh.astype(f32)
    bias = (b_ih + b_hh).astype(f32)          # [256]

    # gate row ranges in torch order (i, f, g, o); psum block order: i, f, o, g
    gr = {"i": (0, 64), "f": (64, 128), "g": (128, 192), "o": (192, 256)}
    order = ("i", "f", "o", "g")

    gh = np.zeros((4, 128, 128), f32)
    gy = np.zeros((4, 24, 128), f32)
    for k, g in enumerate(order):
        r0, r1 = gr[g]
        scale = 2.0 if g == "g" else 1.0      # tanh(g) = 2*sigmoid(2g) - 1
        whT = scale * w_hh[r0:r1, :].T        # [64, 64]
        gh[k, 0:64, 0:64] = whT
        gh[k, 64:128, 64:128] = whT
        wiT = scale * w_ih[r0:r1, :].T        # [11, 64]
        bg = scale * bias[r0:r1]              # [64]
        gy[k, 0, 0:64] = bg
        gy[k, 1:12, 0:64] = wiT
        gy[k, 12, 64:128] = bg
        gy[k, 13:24, 64:128] = wiT

    yc = np.zeros((128, 24), f32)             # ctx part of y_tilde (block-diag)
    yc[0:64, 1:12] = fc_w[:, 0:64].T
    yc[64:128, 13:24] = fc_w[:, 0:64].T
    yb = np.zeros((1, 24), f32)               # ones + fc_b row
    yb[0, 0] = 1.0
    yb[0, 12] = 1.0
    yb[0, 1:12] = fc_b
    yb[0, 13:24] = fc_b

    w_y = fc_w[:, 64:75].T                    # [11, 11]
    yhsel = np.zeros((12, 6 * F, 24), f32)    # (t-in-group, half) selectors
    for t in range(6):
        for h in range(2):
            yhsel[2 * t + h, t * F : (t + 1) * F, 1 + 12 * h : 12 + 12 * h] = w_y
    ytsel = np.zeros((10, 5 * F, 24), f32)
    for t in range(5):
        for h in range(2):
            ytsel[2 * t + h, t * F : (t + 1) * F, 1 + 12 * h : 12 + 12 * h] = w_y

    # prediction operands: accumulate into a [66, cw] psum per half; the
    # stationary places pred p at psum rows p*F (zeros elsewhere accumulate
    # harmlessly, keeping the matmul output base partition at 0).
    ph66 = np.zeros((6, 2, 128, 6 * F), f32)
    for p in range(6):
        for h in range(2):
            ph66[p, h, 64 * h : 64 * h + 64, p * F : (p + 1) * F] = ffin_w[:, 0:64].T
    pc66 = np.zeros((2, 128, 6 * F), f32)     # ctx-part, tiled over 6 preds
    for h in range(2):
        for p in range(6):
            pc66[h, 64 * h : 64 * h + 64, p * F : (p + 1) * F] = ffin_w[:, 64:128].T
    pb66 = np.tile(ffin_b.astype(f32), 6)[None, :]  # [1, 66]

    # pack everything into one [128, WK_NCOL] tensor -> single cast-DMA.
    pk = np.zeros((128, WK_NCOL), f32)
    pk[:, 0:128] = np.eye(128, dtype=f32)
    pk[0, 128:640] = 1.0                                   # ones row
    for k in range(4):
        pk[:, 640 + 128 * k : 768 + 128 * k] = gh[k]
        pk[0:24, 1152 + 128 * k : 1280 + 128 * k] = gy[k]
    pk[:, 1664:1688] = yc
    pk[0:1, 1688:1712] = yb
    for i in range(12):
        pk[0 : 6 * F, 1712 + 24 * i : 1736 + 24 * i] = yhsel[i]
    for i in range(10):
        pk[0 : 5 * F, 2000 + 24 * i : 2024 + 24 * i] = ytsel[i]
    for p in range(6):
        for h in range(2):
            pk[:, 2240 + 66 * (2 * p + h) : 2306 + 66 * (2 * p + h)] = ph66[p, h]
    pk[:, 3032:3098] = pc66[0]
    pk[:, 3098:3164] = pc66[1]
    pk[0:1, 3164:3230] = pb66
    return {"wk_all": pk}


def build_program(b_core: int = B_CORE):
    assert b_core == B_CORE
    nc = bacc.Bacc("TRN2", debug=False)

    x_d = nc.dram_tensor("input_encoded", [b_core, S, E], F32, kind="ExternalInput").ap()
    yhA_d = nc.dram_tensor("yhA_T", [6 * F, b_core], F32, kind="ExternalInput").ap()
    yhB_d = nc.dram_tensor("yhB_T", [6 * F, b_core], F32, kind="ExternalInput").ap()
    ytT_d = nc.dram_tensor("ytT_T", [5 * F, b_core], F32, kind="ExternalInput").ap()
    wk_d = nc.dram_tensor("wk_all", [128, WK_NCOL], F32, kind="ExternalInput").ap()
    out_d = nc.dram_tensor("out", [b_core, NPRED, F], F32, kind="ExternalOutput").ap()

    x_flat = x_d.rearrange("b s e -> b (s e)")        # [b_core, 8192]
    out_flat = out_d.rearrange("b p f -> b (p f)")    # [b_core, 66]

    with tile.TileContext(nc) as tc:
        with (
            tc.tile_pool(name="consts", bufs=1) as consts,
            tc.tile_pool(name="xload", bufs=BUFS_X) as xload,
            tc.tile_pool(name="ctxbm", bufs=3) as ctxbm,
            tc.tile_pool(name="grpd", bufs=8) as grpd,
            tc.tile_pool(name="steptmp", bufs=7) as steptmp,
            tc.tile_pool(name="outbm", bufs=3) as outbm,
            tc.tile_pool(name="pgifo", bufs=2, space="PSUM") as pgifo,
            tc.tile_pool(name="pyt", bufs=1, space="PSUM") as pyt,
            tc.tile_pool(name="ppred", bufs=2, space="PSUM") as ppred,
            tc.tile_pool(name="ptrans", bufs=1, space="PSUM") as ptrans,
        ):
            # ---------------- one-time setup: cast-load stationaries + y
            wk = consts.tile([128, WK_NCOL], F16)
            nc.gpsimd.dma_start(out=wk, in_=wk_d)
            ident = wk[:, 0:128]
            ones = wk[0:1, 128:640]
            GH = [wk[:, 640 + 128 * k : 768 + 128 * k] for k in range(4)]
            GY = [wk[0:24, 1152 + 128 * k : 1280 + 128 * k] for k in range(4)]
            YC = wk[:, 1664:1688]
            YB = wk[0:1, 1688:1712]
            YH_SEL = [
                [wk[0 : 6 * F, 1712 + 24 * (2 * t + h) : 1736 + 24 * (2 * t + h)] for h in range(2)]
                for t in range(6)
            ]
            YT_SEL = [
                [wk[0 : 5 * F, 2000 + 24 * (2 * t + h) : 2024 + 24 * (2 * t + h)] for h in range(2)]
                for t in range(5)
            ]
            PH66 = [
                [wk[:, 2240 + 66 * (2 * p + h) : 2306 + 66 * (2 * p + h)] for h in range(2)]
                for p in range(6)
            ]
            PC66 = [wk[:, 3032:3098], wk[:, 3098:3164]]
            PB66 = wk[0:1, 3164:3230]

            yhA_sb = consts.tile([6 * F, b_core], F16)
            nc.gpsimd.dma_start(out=yhA_sb, in_=yhA_d)
            yhB_sb = consts.tile([6 * F, b_core], F16)
            nc.gpsimd.dma_start(out=yhB_sb, in_=yhB_d)
            ytT_sb = consts.tile([5 * F, b_core], F16)
            nc.gpsimd.dma_start(out=ytT_sb, in_=ytT_d)

            # ---------------- per-unit (b-tile) streaming ops
            xt_tiles = {}

            def emit_x_dma(u):
                xt = xload.tile([128, S * E], F16, tag="xt", name=f"xt_{u}")
                xt_tiles[u] = xt
                nc.gpsimd.dma_start(out=xt, in_=x_flat[u * 128 : (u + 1) * 128, :])

            def emit_tree(u, states):
                """In-place mean tree + ctx transpose/copy for b-tile u."""
                g = u // UPG
                upg = u - g * UPG
                half, bt = divmod(upg, UPG // 2)
                cslice = slice(bt * 128, (bt + 1) * 128)
                rrow = slice(half * 64, half * 64 + 64)
                if g not in states:
                    states[g] = alloc_state(g)
                st = states[g]

                xt = xt_tiles.pop(u)
                # level 1 into a fresh half-size tile so the 16KB x slot is
                # freed as soon as the first DVE add retires
                t2 = ctxbm.tile([128, S * E // 2], F16, tag="t2", name=f"t2_{u}")
                nc.vector.tensor_add(t2, xt[:, 0 : S * E // 2], xt[:, S * E // 2 :])
                w = S * E // 4
                while w >= 128:
                    eng = nc.vector if w >= 2048 else nc.gpsimd
                    eng.tensor_add(t2[:, 0:w], t2[:, 0:w], t2[:, w : 2 * w])
                    w //= 2
                cbm = ctxbm.tile([128, E], F16, tag="cbm")
                nc.gpsimd.tensor_add(cbm, t2[:, 0:64], t2[:, 64:128])
                ptc = ptrans.tile([128, 128], F16, tag="ptr", name=f"ptc_{u}")
                nc.tensor.transpose(ptc[:E, :], cbm, ident)
                nc.scalar.activation(
                    st["CTX2"][rrow, cslice], ptc[0:64, 0:128], AF.Copy, scale=1.0 / S
                )

            def alloc_state(g):
                st = {}
                st["CTX2"] = grpd.tile([128, CW], F16, tag="ctx2", name=f"CTX2_{g}")
                b0 = g * 2 * CW
                st["yhA"] = [yhA_sb[:, b0 + h * CW : b0 + (h + 1) * CW] for h in range(2)]
                st["yhB"] = [yhB_sb[:, b0 + h * CW : b0 + (h + 1) * CW] for h in range(2)]
                st["ytT"] = [ytT_sb[:, b0 + h * CW : b0 + (h + 1) * CW] for h in range(2)]
                st["Ys2"] = {}
                return st

            def emit_ctx_terms(g, st):
                """Step-invariant terms (once per group, data-ready)."""
                YcP = pyt.tile([24, CW], F32, tag="ypred", name=f"YcP_{g}")
                nc.tensor.matmul(YcP, YB, ones[0:1, 0:CW], start=True, stop=False)
                nc.tensor.matmul(YcP, YC, st["CTX2"], start=False, stop=True)
                st["ytcS"] = grpd.tile([24, CW], F16, tag="ytcs", name=f"ytcS_{g}")
                nc.scalar.copy(st["ytcS"], YcP)
                st["C2"] = grpd.tile([128, CW], F16, tag="c2", name=f"C2_{g}")
                st["H2"] = grpd.tile([128, CW], F16, tag="h2", name=f"H2_{g}")

            def emit_pred_seed(g, st):
                # pred psum [66, 2*CW] seeded with ctx part + bias; allocated
                # late (t=10) so only ~2 groups hold a PRED tile at once.
                PRED = ppred.tile([66, 2 * CW], F32, tag="pred", name=f"PRED_{g}")
                st["PRED"] = PRED
                # single full-width start=True: psum accumulation-start acts
                # at bank granularity, so per-half starts would clobber the
                # other half's seed
                nc.tensor.matmul(PRED, PB66, ones[0:1, 0 : 2 * CW], start=True,
                                 stop=False, skip_group_check=True)
                for h in range(2):
                    nc.tensor.matmul(PRED[:, h * CW : (h + 1) * CW], PC66[h],
                                     st["CTX2"], start=False, stop=False,
                                     skip_group_check=True)

            def emit_front_y(g, t, st):
                """y-part of y_tilde for step t (hoisted one round early)."""
                if t < 6:
                    ysrc, ysel = st["yhA"], YH_SEL[t]
                elif t < 12:
                    ysrc, ysel = st["yhB"], YH_SEL[t - 6]
                else:
                    ysrc, ysel = st["ytT"], YT_SEL[t - 12]
                Y2 = pyt.tile([24, CW], F32, tag="ypred", name=f"Y2_{g}_{t}")
                nc.tensor.matmul(Y2, ysel[0], ysrc[0], start=True, stop=False)
                nc.tensor.matmul(Y2, ysel[1], ysrc[1], start=False, stop=True)
                Ys2 = steptmp.tile([24, CW], F16, tag="ys2", name=f"Ys2_{g}_{t}")
                nc.vector.tensor_add(Ys2, Y2, st["ytcS"])
                st["Ys2"][t] = Ys2

            def emit_mid(g, t, st):
                """gates + single sigmoid for step t."""
                Ys2 = st["Ys2"].pop(t)
                IFOG = pgifo.tile([128, 4 * CW], F32, tag="gifo", name=f"IFOG_{g}_{t}")
                for gi in range(4):
                    dst = IFOG[:, gi * CW : (gi + 1) * CW]
                    if t == 0:  # h0 == 0: y-part only
                        nc.tensor.matmul(dst, GY[gi], Ys2, start=True, stop=True)
                    else:
                        nc.tensor.matmul(dst, GY[gi], Ys2, start=True, stop=False)
                        nc.tensor.matmul(dst, GH[gi], st["H2"], start=False, stop=True)
                SIG4 = steptmp.tile([128, 4 * CW], F16, tag="sig4", name=f"SIG4_{g}_{t}")
                nc.scalar.activation(SIG4, IFOG, AF.Sigmoid)
                st["_f"] = SIG4

            def emit_back(g, t, st):
                SIG4 = st.pop("_f")
                C2, H2 = st["C2"], st["H2"]
                # tanh(g) = 2*sigmoid(2g) - 1 (4x-mode tensor_scalar)
                TGs = steptmp.tile([128, CW], F16, tag="tgs", name=f"TGs_{g}_{t}")
                nc.vector.tensor_scalar(TGs, SIG4[:, 3 * CW : 4 * CW], 2.0, -1.0,
                                        ALU.mult, ALU.add)
                # c = f*c + i*tanh(g);  h = o * tanh(c)
                if t == 0:  # c0 == 0
                    nc.vector.tensor_mul(C2, SIG4[:, 0:CW], TGs)
                else:
                    TMP = steptmp.tile([128, CW], F16, tag="tmp", name=f"TMP_{g}_{t}")
                    nc.vector.tensor_mul(C2, SIG4[:, CW : 2 * CW], C2)
                    nc.vector.tensor_mul(TMP, SIG4[:, 0:CW], TGs)
                    nc.vector.tensor_add(C2, C2, TMP)
                TCs = steptmp.tile([128, CW], F16, tag="tcs", name=f"TCs_{g}_{t}")
                nc.scalar.activation(TCs, C2, AF.Tanh)
                nc.vector.tensor_mul(H2, SIG4[:, 2 * CW : 3 * CW], TCs)

                # prediction after steps 11..16 -> row slice of PRED psum
                if t >= T - 1:
                    p = t - (T - 1)
                    PRED = st["PRED"]
                    for h in range(2):
                        nc.tensor.matmul(
                            PRED[:, h * CW : (h + 1) * CW],
                            PH66[p][h], H2, start=False, stop=(p == NPRED - 1),
                            skip_group_check=True,
                        )

            def emit_out(g, st):
                PRED = st["PRED"]
                for h in range(2):
                    o66 = grpd.tile([66, CW], F16, tag="o66", name=f"o66_{g}_{h}")
                    nc.scalar.copy(o66, PRED[:, h * CW : (h + 1) * CW])
                    for bt in range(CW // 128):
                        r0 = (g * UPG + half_off(h, bt)) * 128
                        pto = ptrans.tile([128, 128], F16, tag="ptr",
                                          name=f"pto_{g}_{h}_{bt}")
                        nc.tensor.transpose(
                            pto[:, 0:66], o66[:, bt * 128 : (bt + 1) * 128],
                            ident[0:66, 0:66],
                        )
                        obm = outbm.tile([128, 66], F32, tag="obm")
                        nc.scalar.copy(obm, pto[:, 0:66])
                        nc.sync.dma_start(
                            out=out_flat[r0 : r0 + 128, :], in_=obm
                        )

            def half_off(h, bt):
                return h * (UPG // 2) + bt

            # ---------------- virtual-time list schedule
            # Every emission item gets an estimated feasible start time; we
            # emit in that order so each in-order engine queue sees work in
            # the sequence it actually becomes runnable.
            TREE_DELAY = 600.0   # data-arrival -> tree emission
            CTX_LAT = 1200.0
            STEP_LAT = 5200.0     # per-step chain latency estimate

            arr = {u: (u + 1) * EST_UNIT + 500.0 for u in range(NB_TILES)}
            key_tree = {u: arr[u] + TREE_DELAY for u in range(NB_TILES)}
            key_dma = {}
            for u in range(NB_TILES):
                if u < BUFS_X:
                    key_dma[u] = float(u)
                else:
                    # pool-order invariant: dma(u) right after tree(u-BUFS_X)
                    key_dma[u] = key_tree[u - BUFS_X] + 1.0

            items = []
            seq = 0
            def add(key, kind, payload):
                nonlocal seq
                items.append((key, seq, kind, payload))
                seq += 1

            for u in range(NB_TILES):
                add(key_dma[u], "dma", u)
                add(key_tree[u], "tree", u)
            key_ctx = {}
            for g in range(NGROUPS):
                key_ctx[g] = key_tree[g * UPG + UPG - 1] + CTX_LAT
                add(key_ctx[g], "ctx", g)
                for t in range(NSTEP):
                    add(key_ctx[g] + 2000.0 + t * STEP_LAT, "step", (g, t))
                add(key_ctx[g] + 2000.0 + (NSTEP - 1) * STEP_LAT + 1.0,
                    "out", g)

            items.sort(key=lambda it: (it[0], it[1]))

            states = {}
            for key, _s, kind, payload in items:
                if kind == "dma":
                    emit_x_dma(payload)
                elif kind == "tree":
                    emit_tree(payload, states)
                elif kind == "ctx":
                    g = payload
                    emit_ctx_terms(g, states[g])
                    emit_front_y(g, 0, states[g])
                elif kind == "step":
                    g, t = payload
                    st = states[g]
                    if t + 1 < NSTEP:
                        emit_front_y(g, t + 1, st)
                    emit_mid(g, t, st)
                    if t == T - 2:
                        emit_pred_seed(g, st)
                    emit_back(g, t, st)
                elif kind == "out":
                    emit_out(payload, states[payload])

    nc.compile()
    return nc


def shard_inputs(full, b_core):
    """Build per-core in_maps from full inputs (host-side layout prep)."""
    wk = host_prep(
        full["fc_w"], full["fc_b"], full["ffin_w"], full["ffin_b"],
        full["w_ih"], full["w_hh"], full["b_ih"], full["b_hh"],
    )
    in_maps = []
    for i in range(NCORES):
        sl = slice(i * b_core, (i + 1) * b_core)
        yh = full["y_hists"][sl].astype(np.float32)      # [b_core, 12, 11]
        yt = full["y_targs"][sl].astype(np.float32)      # [b_core, 5, 11]
        m = {
            "input_encoded": np.ascontiguousarray(full["input_encoded"][sl]),
            "yhA_T": np.ascontiguousarray(yh[:, 0:6, :].reshape(b_core, 66).T),
            "yhB_T": np.ascontiguousarray(yh[:, 6:12, :].reshape(b_core, 66).T),
            "ytT_T": np.ascontiguousarray(yt.reshape(b_core, 55).T),
        }
        m.update(wk)
        in_maps.append(m)
    return in_maps


def kernel(**inputs) -> np.ndarray:
    full = {k: np.asarray(v, dtype=np.float32) for k, v in inputs.items()}
    b_core = full["input_encoded"].shape[0] // NCORES
    nc = build_program(b_core)
    in_maps = shard_inputs(full, b_core)
    res = bass_utils.run_bass_kernel_spmd(nc, in_maps, core_ids=list(range(NCORES)))
    out = np.concatenate([res.results[i]["out"] for i in range(NCORES)], axis=0)
    return out.astype(np.float32)


# revision 31
# speedup vs baseline: 1.6619x; 1.0142x over previous
"""Trainium2 Bass kernel for nn_Decoder (LSTM decoder with mean-context).

Reference computation (per batch row b):
  context = mean_s input_encoded[b, s, :]                  # [E=64]
  LSTM primed 12 steps on y_hists, then 5 gen steps on y_targs,
  pred = ffin_w @ [h; context] + ffin_b after steps 11..16  # 6 preds of F=11
  out[b] = stack(preds)                                     # [6, 11]

Sharding: pure data-parallel over batch across 8 cores (B=32768 -> 4096/core).

v2.2 design (fp16 data plane, minimal op count):
  - input_encoded streams HBM->SBUF via gpsimd (SWDGE) cast-DMAs fp32->fp16.
  - s-mean = in-place fp16 binary tree on the x tile: the two wide levels on
    DVE (2x packed mode), the narrow levels on gpsimd; the Pool queue (which
    also issues the x DMAs) interleaves tree(u-BUFS_X) -> dma(u) so tile
    reuse never head-of-line blocks the x stream.
  - y_hists/y_targs are transposed on the host (pure layout prep) into
    [66|55, B_CORE] tensors; one cast-DMA each, no on-device transposes.
  - gates: one [128, 4cw] psum (i|f|o|g), ONE sigmoid over all four; the
    g-gate weights are pre-scaled by 2 so tanh(g) = 2*sigmoid(2g)-1 is a
    single 4x-mode DVE tensor_scalar.
  - predictions accumulate in a [66, 2cw] psum seeded with ffin_ctx+bias
    via matmuls; per-step pred matmuls land in row slices; output is a
    single [128, 66] transpose+copy+store per 128-batch block.
  - 8 batch groups of 512 (cw=256); per-round emission is software-
    pipelined: Y2/Ys2 of step t+1 are issued before gates of step t, so
    the in-order PE queue never stalls on the DVE y_tilde add.
"""

import sys

import numpy as np

if "/opt/trn_rl_repo" not in sys.path:
    sys.path.insert(0, "/opt/trn_rl_repo")

import concourse.bass as bass
import concourse.tile as tile
from concourse import bacc
from concourse import mybir
from concourse import bass_utils

F32 = mybir.dt.float32
F16 = mybir.dt.float16
AF = mybir.ActivationFunctionType
ALU = mybir.AluOpType

B, S, E, H, T, F = 32768, 128, 64, 64, 12, 11
NCORES = 8
B_CORE = B // NCORES      # 4096
NSTEP = T + 5             # 17 cell steps
NPRED = 6

CW = 256                       # chunk width (group batch = 2*CW = 512)
NGROUPS = B_CORE // (2 * CW)   # 8
NB_TILES = B_CORE // 128       # 32
UPG = NB_TILES // NGROUPS      # 4 units per group

BUFS_X = 6                # in-flight x tiles

WK_NCOL = 3232  # packed stationary-operand tensor width

# emission pacing estimates (ns) for the static schedule
EST_UNIT = 6700.0   # one b-tile cast-DMA on the DMA engines
EST_ROUND = 5000.0  # one LSTM step round


def host_prep(fc_w, fc_b, ffin_w, ffin_b, w_ih, w_hh, b_ih, b_hh):
    """Build all derived stationary operands in numpy (fp32; cast-loaded)."""
    f32 = np.float32
    fc_w = fc_w.astype(f32)
    ffin_w = ffin_w.astype(f32)
    w_ih = w_ih.astype(f32)
    w_hh = w_hh.astype(f32)
    bias = (b_ih + b_hh).astype(f32)          # [256]

    # gate row ranges in torch order (i, f, g, o); psum block order: i, f, o, g
    gr = {"i": (0, 64), "f": (64, 128), "g": (128, 192), "o": (192, 256)}
    order = ("i", "f", "o", "g")

    gh = np.zeros((4, 128, 128), f32)
    gy = np.zeros((4, 24, 128), f32)
    for k, g in enumerate(order):
        r0, r1 = gr[g]
        scale = 2.0 if g == "g" else 1.0      # tanh(g) = 2*sigmoid(2g) - 1
        whT = scale * w_hh[r0:r1, :].T        # [64, 64]
        gh[k, 0:64, 0:64] = whT
        gh[k, 64:128, 64:128] = whT
        wiT = scale * w_ih[r0:r1, :].T        # [11, 64]
        bg = scale * bias[r0:r1]              # [64]
        gy[k, 0, 0:64] = bg
        gy[k, 1:12, 0:64] = wiT
        gy[k, 12, 64:128] = bg
        gy[k, 13:24, 64:128] = wiT

    yc = np.zeros((128, 24), f32)             # ctx part of y_tilde (block-diag)
    yc[0:64, 1:12] = fc_w[:, 0:64].T
    yc[64:128, 13:24] = fc_w[:, 0:64].T
    yb = np.zeros((1, 24), f32)               # ones + fc_b row
    yb[0, 0] = 1.0
    yb[0, 12] = 1.0
    yb[0, 1:12] = fc_b
    yb[0, 13:24] = fc_b

    w_y = fc_w[:, 64:75].T                    # [11, 11]
    yhsel = np.zeros((12, 6 * F, 24), f32)    # (t-in-group, half) selectors
    for t in range(6):
        for h in range(2):
            yhsel[2 * t + h, t * F : (t + 1) * F, 1 + 12 * h : 12 + 12 * h] = w_y
    ytsel = np.zeros((10, 5 * F, 24), f32)
    for t in range(5):
        for h in range(2):
            ytsel[2 * t + h, t * F : (t + 1) * F, 1 + 12 * h : 12 + 12 * h] = w_y

    # prediction operands: accumulate into a [66, cw] psum per half; the
    # stationary places pred p at psum rows p*F (zeros elsewhere accumulate
    # harmlessly, keeping the matmul output base partition at 0).
    ph66 = np.zeros((6, 2, 128, 6 * F), f32)
    for p in range(6):
        for h in range(2):
            ph66[p, h, 64 * h : 64 * h + 64, p * F : (p + 1) * F] = ffin_w[:, 0:64].T
    pc66 = np.zeros((2, 128, 6 * F), f32)     # ctx-part, tiled over 6 preds
    for h in range(2):
        for p in range(6):
            pc66[h, 64 * h : 64 * h + 64, p * F : (p + 1) * F] = ffin_w[:, 64:128].T
    pb66 = np.tile(ffin_b.astype(f32), 6)[None, :]  # [1, 66]

    # pack everything into one [128, WK_NCOL] tensor -> single cast-DMA.
    pk = np.zeros((128, WK_NCOL), f32)
    pk[:, 0:128] = np.eye(128, dtype=f32)
    pk[0, 128:640] = 1.0                                   # ones row
    for k in range(4):
        pk[:, 640 + 128 * k : 768 + 128 * k] = gh[k]
        pk[0:24, 1152 + 128 * k : 1280 + 128 * k] = gy[k]
    pk[:, 1664:1688] = yc
    pk[0:1, 1688:1712] = yb
    for i in range(12):
        pk[0 : 6 * F, 1712 + 24 * i : 1736 + 24 * i] = yhsel[i]
    for i in range(10):
        pk[0 : 5 * F, 2000 + 24 * i : 2024 + 24 * i] = ytsel[i]
    for p in range(6):
        for h in range(2):
            pk[:, 2240 + 66 * (2 * p + h) : 2306 + 66 * (2 * p + h)] = ph66[p, h]
    pk[:, 3032:3098] = pc66[0]
    pk[:, 3098:3164] = pc66[1]
    pk[0:1, 3164:3230] = pb66
    return {"wk_all": pk}


def build_program(b_core: int = B_CORE):
    assert b_core == B_CORE
    nc = bacc.Bacc("TRN2", debug=False)

    x_d = nc.dram_tensor("input_encoded", [b_core, S, E], F32, kind="ExternalInput").ap()
    yhA_d = nc.dram_tensor("yhA_T", [6 * F, b_core], F32, kind="ExternalInput").ap()
    yhB_d = nc.dram_tensor("yhB_T", [6 * F, b_core], F32, kind="ExternalInput").ap()
    ytT_d = nc.dram_tensor("ytT_T", [5 * F, b_core], F32, kind="ExternalInput").ap()
    wk_d = nc.dram_tensor("wk_all", [128, WK_NCOL], F32, kind="ExternalInput").ap()
    out_d = nc.dram_tensor("out", [b_core, NPRED, F], F32, kind="ExternalOutput").ap()

    x_flat = x_d.rearrange("b s e -> b (s e)")        # [b_core, 8192]
    out_flat = out_d.rearrange("b p f -> b (p f)")    # [b_core, 66]

    with tile.TileContext(nc) as tc:
        with (
            tc.tile_pool(name="consts", bufs=1) as consts,
            tc.tile_pool(name="xload", bufs=BUFS_X) as xload,
            tc.tile_pool(name="ctxbm", bufs=3) as ctxbm,
            tc.tile_pool(name="grpd", bufs=8) as grpd,
            tc.tile_pool(name="steptmp", bufs=7) as steptmp,
            tc.tile_pool(name="outbm", bufs=3) as outbm,
            tc.tile_pool(name="pgifo", bufs=2, space="PSUM") as pgifo,
            tc.tile_pool(name="pyt", bufs=2, space="PSUM") as pyt,
            tc.tile_pool(name="ppred", bufs=1, space="PSUM") as ppred,
            tc.tile_pool(name="ptrans", bufs=1, space="PSUM") as ptrans,
        ):
            # ---------------- one-time setup: cast-load stationaries + y
            wk = consts.tile([128, WK_NCOL], F16)
            nc.gpsimd.dma_start(out=wk, in_=wk_d)
            ident = wk[:, 0:128]
            ones = wk[0:1, 128:640]
            GH = [wk[:, 640 + 128 * k : 768 + 128 * k] for k in range(4)]
            GY = [wk[0:24, 1152 + 128 * k : 1280 + 128 * k] for k in range(4)]
            YC = wk[:, 1664:1688]
            YB = wk[0:1, 1688:1712]
            YH_SEL = [
                [wk[0 : 6 * F, 1712 + 24 * (2 * t + h) : 1736 + 24 * (2 * t + h)] for h in range(2)]
                for t in range(6)
            ]
            YT_SEL = [
                [wk[0 : 5 * F, 2000 + 24 * (2 * t + h) : 2024 + 24 * (2 * t + h)] for h in range(2)]
                for t in range(5)
            ]
            PH66 = [
                [wk[:, 2240 + 66 * (2 * p + h) : 2306 + 66 * (2 * p + h)] for h in range(2)]
                for p in range(6)
            ]
            PC66 = [wk[:, 3032:3098], wk[:, 3098:3164]]
            PB66 = wk[0:1, 3164:3230]

            yhA_sb = consts.tile([6 * F, b_core], F16)
            nc.gpsimd.dma_start(out=yhA_sb, in_=yhA_d)
            yhB_sb = consts.tile([6 * F, b_core], F16)
            nc.gpsimd.dma_start(out=yhB_sb, in_=yhB_d)
            ytT_sb = consts.tile([5 * F, b_core], F16)
            nc.gpsimd.dma_start(out=ytT_sb, in_=ytT_d)

            # ---------------- per-unit (b-tile) streaming ops
            xt_tiles = {}

            def emit_x_dma(u):
                xt = xload.tile([128, S * E], F16, tag="xt", name=f"xt_{u}")
                xt_tiles[u] = xt
                nc.gpsimd.dma_start(out=xt, in_=x_flat[u * 128 : (u + 1) * 128, :])

            def emit_tree(u, states):
                """In-place mean tree + ctx transpose/copy for b-tile u."""
                g = u // UPG
                upg = u - g * UPG
                half, bt = divmod(upg, UPG // 2)
                cslice = slice(bt * 128, (bt + 1) * 128)
                rrow = slice(half * 64, half * 64 + 64)
                if g not in states:
                    states[g] = alloc_state(g)
                st = states[g]

                xt = xt_tiles.pop(u)
                # level 1 into a fresh half-size tile so the 16KB x slot is
                # freed as soon as the first DVE add retires
                t2 = ctxbm.tile([128, S * E // 2], F16, tag="t2", name=f"t2_{u}")
                nc.vector.tensor_add(t2, xt[:, 0 : S * E // 2], xt[:, S * E // 2 :])
                w = S * E // 4
                while w >= 128:
                    eng = nc.vector if (w >= 4096 or (w >= 2048 and u % 2 == 0)) else nc.gpsimd
                    eng.tensor_add(t2[:, 0:w], t2[:, 0:w], t2[:, w : 2 * w])
                    w //= 2
                cbm = ctxbm.tile([128, E], F16, tag="cbm")
                nc.gpsimd.tensor_add(cbm, t2[:, 0:64], t2[:, 64:128])
                ptc = ptrans.tile([128, 128], F16, tag="ptr", name=f"ptc_{u}")
                nc.tensor.transpose(ptc[:E, :], cbm, ident)
                nc.scalar.activation(
                    st["CTX2"][rrow, cslice], ptc[0:64, 0:128], AF.Copy, scale=1.0 / S
                )

            def alloc_state(g):
                st = {}
                st["CTX2"] = grpd.tile([128, CW], F16, tag="ctx2", name=f"CTX2_{g}")
                b0 = g * 2 * CW
                st["yhA"] = [yhA_sb[:, b0 + h * CW : b0 + (h + 1) * CW] for h in range(2)]
                st["yhB"] = [yhB_sb[:, b0 + h * CW : b0 + (h + 1) * CW] for h in range(2)]
                st["ytT"] = [ytT_sb[:, b0 + h * CW : b0 + (h + 1) * CW] for h in range(2)]
                st["Ys2"] = {}
                return st

            def emit_ctx_terms(g, st):
                """Step-invariant terms (once per group, data-ready)."""
                YcP = pyt.tile([24, CW], F32, tag="ypred", name=f"YcP_{g}")
                nc.tensor.matmul(YcP, YB, ones[0:1, 0:CW], start=True, stop=False)
                nc.tensor.matmul(YcP, YC, st["CTX2"], start=False, stop=True)
                st["ytcS"] = grpd.tile([24, CW], F16, tag="ytcs", name=f"ytcS_{g}")
                nc.scalar.copy(st["ytcS"], YcP)
                st["C2"] = grpd.tile([128, CW], F16, tag="c2", name=f"C2_{g}")
                st["H2"] = grpd.tile([128, CW], F16, tag="h2", name=f"H2_{g}")

            def emit_pred_seed(g, st):
                # pred psum [66, 2*CW] seeded with ctx part + bias; allocated
                # late (t=10) so only ~2 groups hold a PRED tile at once.
                PRED = ppred.tile([66, 2 * CW], F32, tag="pred", name=f"PRED_{g}")
                st["PRED"] = PRED
                # single full-width start=True: psum accumulation-start acts
                # at bank granularity, so per-half starts would clobber the
                # other half's seed
                nc.tensor.matmul(PRED, PB66, ones[0:1, 0 : 2 * CW], start=True,
                                 stop=False, skip_group_check=True)
                for h in range(2):
                    nc.tensor.matmul(PRED[:, h * CW : (h + 1) * CW], PC66[h],
                                     st["CTX2"], start=False, stop=False,
                                     skip_group_check=True)

            def emit_front_y(g, t, st):
                """y-part of y_tilde for step t (hoisted one round early)."""
                if t < 6:
                    ysrc, ysel = st["yhA"], YH_SEL[t]
                elif t < 12:
                    ysrc, ysel = st["yhB"], YH_SEL[t - 6]
                else:
                    ysrc, ysel = st["ytT"], YT_SEL[t - 12]
                Y2 = pyt.tile([24, CW], F32, tag="ypred", name=f"Y2_{g}_{t}")
                nc.tensor.matmul(Y2, ysel[0], ysrc[0], start=True, stop=False)
                nc.tensor.matmul(Y2, ysel[1], ysrc[1], start=False, stop=True)
                Ys2 = steptmp.tile([24, CW], F16, tag="ys2", name=f"Ys2_{g}_{t}")
                nc.vector.tensor_add(Ys2, Y2, st["ytcS"])
                st["Ys2"][t] = Ys2

            def emit_mid(g, t, st):
                """gates + single sigmoid for step t."""
                Ys2 = st["Ys2"].pop(t)
                IFOG = pgifo.tile([128, 4 * CW], F32, tag="gifo", name=f"IFOG_{g}_{t}")
                for gi in range(4):
                    dst = IFOG[:, gi * CW : (gi + 1) * CW]
                    if t == 0:  # h0 == 0: y-part only
                        nc.tensor.matmul(dst, GY[gi], Ys2, start=True, stop=True)
                    else:
                        nc.tensor.matmul(dst, GY[gi], Ys2, start=True, stop=False)
                        nc.tensor.matmul(dst, GH[gi], st["H2"], start=False, stop=True)
                SIG4 = steptmp.tile([128, 4 * CW], F16, tag="sig4", name=f"SIG4_{g}_{t}")
                nc.scalar.activation(SIG4, IFOG, AF.Sigmoid)
                st["_f"] = SIG4

            def emit_back(g, t, st):
                SIG4 = st.pop("_f")
                C2, H2 = st["C2"], st["H2"]
                # tanh(g) = 2*sigmoid(2g) - 1 (4x-mode tensor_scalar)
                TGs = steptmp.tile([128, CW], F16, tag="tgs", name=f"TGs_{g}_{t}")
                nc.vector.tensor_scalar(TGs, SIG4[:, 3 * CW : 4 * CW], 2.0, -1.0,
                                        ALU.mult, ALU.add)
                # c = f*c + i*tanh(g);  h = o * tanh(c)
                if t == 0:  # c0 == 0
                    nc.vector.tensor_mul(C2, SIG4[:, 0:CW], TGs)
                else:
                    TMP = steptmp.tile([128, CW], F16, tag="tmp", name=f"TMP_{g}_{t}")
                    nc.vector.tensor_mul(C2, SIG4[:, CW : 2 * CW], C2)
                    nc.vector.tensor_mul(TMP, SIG4[:, 0:CW], TGs)
                    nc.vector.tensor_add(C2, C2, TMP)
                TCs = steptmp.tile([128, CW], F16, tag="tcs", name=f"TCs_{g}_{t}")
                nc.scalar.activation(TCs, C2, AF.Tanh)
                nc.vector.tensor_mul(H2, SIG4[:, 2 * CW : 3 * CW], TCs)

                # prediction after steps 11..16 -> row slice of PRED psum
                if t >= T - 1:
                    p = t - (T - 1)
                    PRED = st["PRED"]
                    for h in range(2):
                        nc.tensor.matmul(
                            PRED[:, h * CW : (h + 1) * CW],
                            PH66[p][h], H2, start=False, stop=(p == NPRED - 1),
                            skip_group_check=True,
                        )

            def emit_out(g, st):
                PRED = st["PRED"]
                for h in range(2):
                    o66 = grpd.tile([66, CW], F16, tag="o66", name=f"o66_{g}_{h}")
                    nc.scalar.copy(o66, PRED[:, h * CW : (h + 1) * CW])
                    for bt in range(CW // 128):
                        r0 = (g * UPG + half_off(h, bt)) * 128
                        pto = ptrans.tile([128, 128], F16, tag="ptr",
                                          name=f"pto_{g}_{h}_{bt}")
                        nc.tensor.transpose(
                            pto[:, 0:66], o66[:, bt * 128 : (bt + 1) * 128],
                            ident[0:66, 0:66],
                        )
                        obm = outbm.tile([128, 66], F32, tag="obm")
                        nc.scalar.copy(obm, pto[:, 0:66])
                        nc.sync.dma_start(
                            out=out_flat[r0 : r0 + 128, :], in_=obm
                        )

            def half_off(h, bt):
                return h * (UPG // 2) + bt

            # ---------------- virtual-time list schedule
            # Every emission item gets an estimated feasible start time; we
            # emit in that order so each in-order engine queue sees work in
            # the sequence it actually becomes runnable.
            TREE_DELAY = 600.0   # data-arrival -> tree emission
            CTX_LAT = 1200.0
            STEP_LAT = 5200.0     # per-step chain latency estimate

            arr = {u: (u + 1) * EST_UNIT + 500.0 for u in range(NB_TILES)}
            key_tree = {u: arr[u] + TREE_DELAY for u in range(NB_TILES)}
            key_dma = {}
            for u in range(NB_TILES):
                if u < BUFS_X:
                    key_dma[u] = float(u)
                else:
                    # pool-order invariant: dma(u) right after tree(u-BUFS_X)
                    key_dma[u] = key_tree[u - BUFS_X] + 1.0

            items = []
            seq = 0
            def add(key, kind, payload):
                nonlocal seq
                items.append((key, seq, kind, payload))
                seq += 1

            for u in range(NB_TILES):
                add(key_dma[u], "dma", u)
                add(key_tree[u], "tree", u)
            key_ctx = {}
            for g in range(NGROUPS):
                key_ctx[g] = key_tree[g * UPG + UPG - 1] + CTX_LAT
                add(key_ctx[g], "ctx", g)
                for t in range(NSTEP):
                    add(key_ctx[g] + 2000.0 + t * STEP_LAT, "step", (g, t))
                add(key_ctx[g] + 2000.0 + (NSTEP - 1) * STEP_LAT + 1.0,
                    "out", g)

            items.sort(key=lambda it: (it[0], it[1]))

            states = {}
            for key, _s, kind, payload in items:
                if kind == "dma":
                    emit_x_dma(payload)
                elif kind == "tree":
                    emit_tree(payload, states)
                elif kind == "ctx":
                    g = payload
                    emit_ctx_terms(g, states[g])
                    emit_front_y(g, 0, states[g])
                elif kind == "step":
                    g, t = payload
                    st = states[g]
                    if t + 1 < NSTEP:
                        emit_front_y(g, t + 1, st)
                    emit_mid(g, t, st)
                    if t == T - 2:
                        emit_pred_seed(g, st)
                    emit_back(g, t, st)
                elif kind == "out":
                    emit_out(payload, states[payload])

    nc.compile()
    return nc


def shard_inputs(full, b_core):
    """Build per-core in_maps from full inputs (host-side layout prep)."""
    wk = host_prep(
        full["fc_w"], full["fc_b"], full["ffin_w"], full["ffin_b"],
        full["w_ih"], full["w_hh"], full["b_ih"], full["b_hh"],
    )
    in_maps = []
    for i in range(NCORES):
        sl = slice(i * b_core, (i + 1) * b_core)
        yh = full["y_hists"][sl].astype(np.float32)      # [b_core, 12, 11]
        yt = full["y_targs"][sl].astype(np.float32)      # [b_core, 5, 11]
        m = {
            "input_encoded": np.ascontiguousarray(full["input_encoded"][sl]),
            "yhA_T": np.ascontiguousarray(yh[:, 0:6, :].reshape(b_core, 66).T),
            "yhB_T": np.ascontiguousarray(yh[:, 6:12, :].reshape(b_core, 66).T),
            "ytT_T": np.ascontiguousarray(yt.reshape(b_core, 55).T),
        }
        m.update(wk)
        in_maps.append(m)
    return in_maps


def kernel(**inputs) -> np.ndarray:
    full = {k: np.asarray(v, dtype=np.float32) for k, v in inputs.items()}
    b_core = full["input_encoded"].shape[0] // NCORES
    nc = build_program(b_core)
    in_maps = shard_inputs(full, b_core)
    res = bass_utils.run_bass_kernel_spmd(nc, in_maps, core_ids=list(range(NCORES)))
    out = np.concatenate([res.results[i]["out"] for i in range(NCORES)], axis=0)
    return out.astype(np.float32)


# revision 39
# speedup vs baseline: 1.6717x; 1.0059x over previous
"""Trainium2 Bass kernel for nn_Decoder (LSTM decoder with mean-context).

Reference computation (per batch row b):
  context = mean_s input_encoded[b, s, :]                  # [E=64]
  LSTM primed 12 steps on y_hists, then 5 gen steps on y_targs,
  pred = ffin_w @ [h; context] + ffin_b after steps 11..16  # 6 preds of F=11
  out[b] = stack(preds)                                     # [6, 11]

Sharding: pure data-parallel over batch across 8 cores (B=32768 -> 4096/core).

v2.2 design (fp16 data plane, minimal op count):
  - input_encoded streams HBM->SBUF via gpsimd (SWDGE) cast-DMAs fp32->fp16.
  - s-mean = in-place fp16 binary tree on the x tile: the two wide levels on
    DVE (2x packed mode), the narrow levels on gpsimd; the Pool queue (which
    also issues the x DMAs) interleaves tree(u-BUFS_X) -> dma(u) so tile
    reuse never head-of-line blocks the x stream.
  - y_hists/y_targs are transposed on the host (pure layout prep) into
    [66|55, B_CORE] tensors; one cast-DMA each, no on-device transposes.
  - gates: one [128, 4cw] psum (i|f|o|g), ONE sigmoid over all four; the
    g-gate weights are pre-scaled by 2 so tanh(g) = 2*sigmoid(2g)-1 is a
    single 4x-mode DVE tensor_scalar.
  - predictions accumulate in a [66, 2cw] psum seeded with ffin_ctx+bias
    via matmuls; per-step pred matmuls land in row slices; output is a
    single [128, 66] transpose+copy+store per 128-batch block.
  - 8 batch groups of 512 (cw=256); per-round emission is software-
    pipelined: Y2/Ys2 of step t+1 are issued before gates of step t, so
    the in-order PE queue never stalls on the DVE y_tilde add.
"""

import sys

import numpy as np

if "/opt/trn_rl_repo" not in sys.path:
    sys.path.insert(0, "/opt/trn_rl_repo")

import concourse.bass as bass
import concourse.tile as tile
from concourse import bacc
from concourse import mybir
from concourse import bass_utils

F32 = mybir.dt.float32
F16 = mybir.dt.float16
AF = mybir.ActivationFunctionType
ALU = mybir.AluOpType

B, S, E, H, T, F = 32768, 128, 64, 64, 12, 11
NCORES = 8
B_CORE = B // NCORES      # 4096
NSTEP = T + 5             # 17 cell steps
NPRED = 6

CW = 256                       # chunk width (group batch = 2*CW = 512)
NGROUPS = B_CORE // (2 * CW)   # 8
NB_TILES = B_CORE // 128       # 32
UPG = NB_TILES // NGROUPS      # 4 units per group

BUFS_X = 6                # in-flight x tiles

WK_NCOL = 3232  # packed stationary-operand tensor width

# emission pacing estimates (ns) for the static schedule
EST_UNIT = 6700.0   # one b-tile cast-DMA on the DMA engines
EST_ROUND = 5000.0  # one LSTM step round


def host_prep(fc_w, fc_b, ffin_w, ffin_b, w_ih, w_hh, b_ih, b_hh):
    """Build all derived stationary operands in numpy (fp32; cast-loaded)."""
    f32 = np.float32
    fc_w = fc_w.astype(f32)
    ffin_w = ffin_w.astype(f32)
    w_ih = w_ih.astype(f32)
    w_hh = w_hh.astype(f32)
    bias = (b_ih + b_hh).astype(f32)          # [256]

    # gate row ranges in torch order (i, f, g, o); psum block order: i, f, o, g
    gr = {"i": (0, 64), "f": (64, 128), "g": (128, 192), "o": (192, 256)}
    order = ("i", "f", "o", "g")

    gh = np.zeros((4, 128, 128), f32)
    gy = np.zeros((4, 24, 128), f32)
    for k, g in enumerate(order):
        r0, r1 = gr[g]
        scale = 2.0 if g == "g" else 1.0      # tanh(g) = 2*sigmoid(2g) - 1
        whT = scale * w_hh[r0:r1, :].T        # [64, 64]
        gh[k, 0:64, 0:64] = whT
        gh[k, 64:128, 64:128] = whT
        wiT = scale * w_ih[r0:r1, :].T        # [11, 64]
        bg = scale * bias[r0:r1]              # [64]
        gy[k, 0, 0:64] = bg
        gy[k, 1:12, 0:64] = wiT
        gy[k, 12, 64:128] = bg
        gy[k, 13:24, 64:128] = wiT

    yc = np.zeros((128, 24), f32)             # ctx part of y_tilde (block-diag)
    yc[0:64, 1:12] = fc_w[:, 0:64].T
    yc[64:128, 13:24] = fc_w[:, 0:64].T
    yb = np.zeros((1, 24), f32)               # ones + fc_b row
    yb[0, 0] = 1.0
    yb[0, 12] = 1.0
    yb[0, 1:12] = fc_b
    yb[0, 13:24] = fc_b

    w_y = fc_w[:, 64:75].T                    # [11, 11]
    yhsel = np.zeros((12, 6 * F, 24), f32)    # (t-in-group, half) selectors
    for t in range(6):
        for h in range(2):
            yhsel[2 * t + h, t * F : (t + 1) * F, 1 + 12 * h : 12 + 12 * h] = w_y
    ytsel = np.zeros((10, 5 * F, 24), f32)
    for t in range(5):
        for h in range(2):
            ytsel[2 * t + h, t * F : (t + 1) * F, 1 + 12 * h : 12 + 12 * h] = w_y

    # prediction operands: accumulate into a [66, cw] psum per half; the
    # stationary places pred p at psum rows p*F (zeros elsewhere accumulate
    # harmlessly, keeping the matmul output base partition at 0).
    ph66 = np.zeros((6, 2, 128, 6 * F), f32)
    for p in range(6):
        for h in range(2):
            ph66[p, h, 64 * h : 64 * h + 64, p * F : (p + 1) * F] = ffin_w[:, 0:64].T
    pc66 = np.zeros((2, 128, 6 * F), f32)     # ctx-part, tiled over 6 preds
    for h in range(2):
        for p in range(6):
            pc66[h, 64 * h : 64 * h + 64, p * F : (p + 1) * F] = ffin_w[:, 64:128].T
    pb66 = np.tile(ffin_b.astype(f32), 6)[None, :]  # [1, 66]

    # pack everything into one [128, WK_NCOL] tensor -> single cast-DMA.
    pk = np.zeros((128, WK_NCOL), f32)
    pk[:, 0:128] = np.eye(128, dtype=f32)
    pk[0, 128:640] = 1.0                                   # ones row
    for k in range(4):
        pk[:, 640 + 128 * k : 768 + 128 * k] = gh[k]
        pk[0:24, 1152 + 128 * k : 1280 + 128 * k] = gy[k]
    pk[:, 1664:1688] = yc
    pk[0:1, 1688:1712] = yb
    for i in range(12):
        pk[0 : 6 * F, 1712 + 24 * i : 1736 + 24 * i] = yhsel[i]
    for i in range(10):
        pk[0 : 5 * F, 2000 + 24 * i : 2024 + 24 * i] = ytsel[i]
    for p in range(6):
        for h in range(2):
            pk[:, 2240 + 66 * (2 * p + h) : 2306 + 66 * (2 * p + h)] = ph66[p, h]
    pk[:, 3032:3098] = pc66[0]
    pk[:, 3098:3164] = pc66[1]
    pk[0:1, 3164:3230] = pb66
    return {"wk_all": pk}


def build_program(b_core: int = B_CORE):
    assert b_core == B_CORE
    nc = bacc.Bacc("TRN2", debug=False)

    x_d = nc.dram_tensor("input_encoded", [b_core, S, E], F32, kind="ExternalInput").ap()
    yhA_d = nc.dram_tensor("yhA_T", [6 * F, b_core], F32, kind="ExternalInput").ap()
    yhB_d = nc.dram_tensor("yhB_T", [6 * F, b_core], F32, kind="ExternalInput").ap()
    ytT_d = nc.dram_tensor("ytT_T", [5 * F, b_core], F32, kind="ExternalInput").ap()
    wk_d = nc.dram_tensor("wk_all", [128, WK_NCOL], F32, kind="ExternalInput").ap()
    out_d = nc.dram_tensor("out", [b_core, NPRED, F], F32, kind="ExternalOutput").ap()

    x_flat = x_d.rearrange("b s e -> b (s e)")        # [b_core, 8192]
    out_flat = out_d.rearrange("b p f -> b (p f)")    # [b_core, 66]

    with tile.TileContext(nc) as tc:
        with (
            tc.tile_pool(name="consts", bufs=1) as consts,
            tc.tile_pool(name="xload", bufs=BUFS_X) as xload,
            tc.tile_pool(name="ctxbm", bufs=3) as ctxbm,
            tc.tile_pool(name="grpd", bufs=8) as grpd,
            tc.tile_pool(name="steptmp", bufs=7) as steptmp,
            tc.tile_pool(name="outbm", bufs=3) as outbm,
            tc.tile_pool(name="pgifo", bufs=2, space="PSUM") as pgifo,
            tc.tile_pool(name="pyt", bufs=2, space="PSUM") as pyt,
            tc.tile_pool(name="ppred", bufs=1, space="PSUM") as ppred,
            tc.tile_pool(name="ptrans", bufs=1, space="PSUM") as ptrans,
        ):
            # ---------------- one-time setup: cast-load stationaries + y
            wk = consts.tile([128, WK_NCOL], F16)
            nc.gpsimd.dma_start(out=wk, in_=wk_d)
            ident = wk[:, 0:128]
            ones = wk[0:1, 128:640]
            GH = [wk[:, 640 + 128 * k : 768 + 128 * k] for k in range(4)]
            GY = [wk[0:24, 1152 + 128 * k : 1280 + 128 * k] for k in range(4)]
            YC = wk[:, 1664:1688]
            YB = wk[0:1, 1688:1712]
            YH_SEL = [
                [wk[0 : 6 * F, 1712 + 24 * (2 * t + h) : 1736 + 24 * (2 * t + h)] for h in range(2)]
                for t in range(6)
            ]
            YT_SEL = [
                [wk[0 : 5 * F, 2000 + 24 * (2 * t + h) : 2024 + 24 * (2 * t + h)] for h in range(2)]
                for t in range(5)
            ]
            PH66 = [
                [wk[:, 2240 + 66 * (2 * p + h) : 2306 + 66 * (2 * p + h)] for h in range(2)]
                for p in range(6)
            ]
            PC66 = [wk[:, 3032:3098], wk[:, 3098:3164]]
            PB66 = wk[0:1, 3164:3230]

            yhA_sb = consts.tile([6 * F, b_core], F16)
            nc.gpsimd.dma_start(out=yhA_sb, in_=yhA_d)
            yhB_sb = consts.tile([6 * F, b_core], F16)
            nc.gpsimd.dma_start(out=yhB_sb, in_=yhB_d)
            ytT_sb = consts.tile([5 * F, b_core], F16)
            nc.gpsimd.dma_start(out=ytT_sb, in_=ytT_d)

            # ---------------- per-unit (b-tile) streaming ops
            xt_tiles = {}

            def emit_x_dma(u):
                xt = xload.tile([128, S * E], F16, tag="xt", name=f"xt_{u}")
                xt_tiles[u] = xt
                nc.gpsimd.dma_start(out=xt, in_=x_flat[u * 128 : (u + 1) * 128, :])

            def emit_tree(u, states):
                """In-place mean tree + ctx transpose/copy for b-tile u."""
                g = u // UPG
                upg = u - g * UPG
                half, bt = divmod(upg, UPG // 2)
                cslice = slice(bt * 128, (bt + 1) * 128)
                rrow = slice(half * 64, half * 64 + 64)
                if g not in states:
                    states[g] = alloc_state(g)
                st = states[g]

                xt = xt_tiles.pop(u)
                # level 1 into a fresh half-size tile so the 16KB x slot is
                # freed as soon as the first DVE add retires
                t2 = ctxbm.tile([128, S * E // 2], F16, tag="t2", name=f"t2_{u}")
                nc.vector.tensor_add(t2, xt[:, 0 : S * E // 2], xt[:, S * E // 2 :])
                w = S * E // 4
                while w >= 128:
                    eng = nc.vector if (w >= 4096 or (w >= 2048 and u % 2 == 0)) else nc.gpsimd
                    eng.tensor_add(t2[:, 0:w], t2[:, 0:w], t2[:, w : 2 * w])
                    w //= 2
                cbm = ctxbm.tile([128, E], F16, tag="cbm")
                nc.gpsimd.tensor_add(cbm, t2[:, 0:64], t2[:, 64:128])
                ptc = ptrans.tile([128, 128], F16, tag="ptr", name=f"ptc_{u}")
                nc.tensor.transpose(ptc[:E, :], cbm, ident)
                nc.scalar.activation(
                    st["CTX2"][rrow, cslice], ptc[0:64, 0:128], AF.Copy, scale=1.0 / S
                )

            def alloc_state(g):
                st = {}
                st["CTX2"] = grpd.tile([128, CW], F16, tag="ctx2", name=f"CTX2_{g}")
                b0 = g * 2 * CW
                st["yhA"] = [yhA_sb[:, b0 + h * CW : b0 + (h + 1) * CW] for h in range(2)]
                st["yhB"] = [yhB_sb[:, b0 + h * CW : b0 + (h + 1) * CW] for h in range(2)]
                st["ytT"] = [ytT_sb[:, b0 + h * CW : b0 + (h + 1) * CW] for h in range(2)]
                st["Ys2"] = {}
                return st

            def emit_ctx_terms(g, st):
                """Step-invariant terms (once per group, data-ready)."""
                YcP = pyt.tile([24, CW], F32, tag="ypred", name=f"YcP_{g}")
                nc.tensor.matmul(YcP, YB, ones[0:1, 0:CW], start=True, stop=False)
                nc.tensor.matmul(YcP, YC, st["CTX2"], start=False, stop=True)
                st["ytcS"] = grpd.tile([24, CW], F16, tag="ytcs", name=f"ytcS_{g}")
                nc.scalar.copy(st["ytcS"], YcP)
                st["C2"] = grpd.tile([128, CW], F16, tag="c2", name=f"C2_{g}")
                st["H2"] = grpd.tile([128, CW], F16, tag="h2", name=f"H2_{g}")

            def emit_pred_seed(g, st):
                # pred psum [66, 2*CW] seeded with ctx part + bias; allocated
                # late (t=10) so only ~2 groups hold a PRED tile at once.
                PRED = ppred.tile([66, 2 * CW], F32, tag="pred", name=f"PRED_{g}")
                st["PRED"] = PRED
                # single full-width start=True: psum accumulation-start acts
                # at bank granularity, so per-half starts would clobber the
                # other half's seed
                nc.tensor.matmul(PRED, PB66, ones[0:1, 0 : 2 * CW], start=True,
                                 stop=False, skip_group_check=True)
                for h in range(2):
                    nc.tensor.matmul(PRED[:, h * CW : (h + 1) * CW], PC66[h],
                                     st["CTX2"], start=False, stop=False,
                                     skip_group_check=True)

            def emit_front_y(g, t, st):
                """y-part of y_tilde for step t (hoisted one round early)."""
                if t < 6:
                    ysrc, ysel = st["yhA"], YH_SEL[t]
                elif t < 12:
                    ysrc, ysel = st["yhB"], YH_SEL[t - 6]
                else:
                    ysrc, ysel = st["ytT"], YT_SEL[t - 12]
                Y2 = pyt.tile([24, CW], F32, tag="ypred", name=f"Y2_{g}_{t}")
                nc.tensor.matmul(Y2, ysel[0], ysrc[0], start=True, stop=False)
                nc.tensor.matmul(Y2, ysel[1], ysrc[1], start=False, stop=True)
                Ys2 = steptmp.tile([24, CW], F16, tag="ys2", name=f"Ys2_{g}_{t}")
                nc.vector.tensor_add(Ys2, Y2, st["ytcS"])
                st["Ys2"][t] = Ys2

            def emit_mid(g, t, st):
                """gates + single sigmoid for step t."""
                Ys2 = st["Ys2"].pop(t)
                IFOG = pgifo.tile([128, 4 * CW], F32, tag="gifo", name=f"IFOG_{g}_{t}")
                for gi in range(4):
                    dst = IFOG[:, gi * CW : (gi + 1) * CW]
                    if t == 0:  # h0 == 0: y-part only
                        nc.tensor.matmul(dst, GY[gi], Ys2, start=True, stop=True)
                    else:
                        nc.tensor.matmul(dst, GY[gi], Ys2, start=True, stop=False)
                        nc.tensor.matmul(dst, GH[gi], st["H2"], start=False, stop=True)
                SIG4 = steptmp.tile([128, 4 * CW], F16, tag="sig4", name=f"SIG4_{g}_{t}")
                nc.scalar.activation(SIG4, IFOG, AF.Sigmoid)
                st["_f"] = SIG4

            def emit_back(g, t, st):
                SIG4 = st.pop("_f")
                C2, H2 = st["C2"], st["H2"]
                # tanh(g) = 2*sigmoid(2g) - 1 (4x-mode tensor_scalar)
                TGs = steptmp.tile([128, CW], F16, tag="tgs", name=f"TGs_{g}_{t}")
                nc.vector.tensor_scalar(TGs, SIG4[:, 3 * CW : 4 * CW], 2.0, -1.0,
                                        ALU.mult, ALU.add)
                # c = f*c + i*tanh(g);  h = o * tanh(c)
                if t == 0:  # c0 == 0
                    nc.vector.tensor_mul(C2, SIG4[:, 0:CW], TGs)
                else:
                    TMP = steptmp.tile([128, CW], F16, tag="tmp", name=f"TMP_{g}_{t}")
                    nc.vector.tensor_mul(C2, SIG4[:, CW : 2 * CW], C2)
                    nc.vector.tensor_mul(TMP, SIG4[:, 0:CW], TGs)
                    nc.vector.tensor_add(C2, C2, TMP)
                TCs = steptmp.tile([128, CW], F16, tag="tcs", name=f"TCs_{g}_{t}")
                nc.scalar.activation(TCs, C2, AF.Tanh)
                nc.vector.tensor_mul(H2, SIG4[:, 2 * CW : 3 * CW], TCs)

                # prediction after steps 11..16 -> row slice of PRED psum
                if t >= T - 1:
                    p = t - (T - 1)
                    PRED = st["PRED"]
                    for h in range(2):
                        nc.tensor.matmul(
                            PRED[:, h * CW : (h + 1) * CW],
                            PH66[p][h], H2, start=False, stop=(p == NPRED - 1),
                            skip_group_check=True,
                        )

            def emit_out(g, st):
                PRED = st["PRED"]
                for h in range(2):
                    o66 = grpd.tile([66, CW], F16, tag="o66", name=f"o66_{g}_{h}")
                    nc.scalar.copy(o66, PRED[:, h * CW : (h + 1) * CW])
                    for bt in range(CW // 128):
                        r0 = (g * UPG + half_off(h, bt)) * 128
                        pto = ptrans.tile([128, 128], F16, tag="ptr",
                                          name=f"pto_{g}_{h}_{bt}")
                        nc.tensor.transpose(
                            pto[:, 0:66], o66[:, bt * 128 : (bt + 1) * 128],
                            ident[0:66, 0:66],
                        )
                        obm = outbm.tile([128, 66], F32, tag="obm")
                        nc.scalar.copy(obm, pto[:, 0:66])
                        nc.sync.dma_start(
                            out=out_flat[r0 : r0 + 128, :], in_=obm
                        )

            def half_off(h, bt):
                return h * (UPG // 2) + bt

            # ---------------- virtual-time list schedule
            # Every emission item gets an estimated feasible start time; we
            # emit in that order so each in-order engine queue sees work in
            # the sequence it actually becomes runnable.
            TREE_DELAY = 600.0   # data-arrival -> tree emission
            CTX_LAT = 1200.0
            STEP_LAT = 5000.0     # per-step chain latency estimate

            arr = {u: (u + 1) * EST_UNIT + 500.0 for u in range(NB_TILES)}
            key_tree = {u: arr[u] + TREE_DELAY for u in range(NB_TILES)}
            key_dma = {}
            for u in range(NB_TILES):
                if u < BUFS_X:
                    key_dma[u] = float(u)
                else:
                    # pool-order invariant: dma(u) right after tree(u-BUFS_X)
                    key_dma[u] = key_tree[u - BUFS_X] + 1.0

            items = []
            seq = 0
            def add(key, kind, payload):
                nonlocal seq
                items.append((key, seq, kind, payload))
                seq += 1

            for u in range(NB_TILES):
                add(key_dma[u], "dma", u)
                add(key_tree[u], "tree", u)
            key_ctx = {}
            for g in range(NGROUPS):
                key_ctx[g] = key_tree[g * UPG + UPG - 1] + CTX_LAT
                add(key_ctx[g], "ctx", g)
                for t in range(NSTEP):
                    add(key_ctx[g] + 2000.0 + t * STEP_LAT, "step", (g, t))
                add(key_ctx[g] + 2000.0 + (NSTEP - 1) * STEP_LAT + 1.0,
                    "out", g)

            items.sort(key=lambda it: (it[0], it[1]))

            states = {}
            for key, _s, kind, payload in items:
                if kind == "dma":
                    emit_x_dma(payload)
                elif kind == "tree":
                    emit_tree(payload, states)
                elif kind == "ctx":
                    g = payload
                    emit_ctx_terms(g, states[g])
                    emit_front_y(g, 0, states[g])
                elif kind == "step":
                    g, t = payload
                    st = states[g]
                    if t + 1 < NSTEP:
                        emit_front_y(g, t + 1, st)
                    emit_mid(g, t, st)
                    if t == T - 2:
                        emit_pred_seed(g, st)
                    emit_back(g, t, st)
                elif kind == "out":
                    emit_out(payload, states[payload])

    nc.compile()
    return nc


def shard_inputs(full, b_core):
    """Build per-core in_maps from full inputs (host-side layout prep)."""
    wk = host_prep(
        full["fc_w"], full["fc_b"], full["ffin_w"], full["ffin_b"],
        full["w_ih"], full["w_hh"], full["b_ih"], full["b_hh"],
    )
    in_maps = []
    for i in range(NCORES):
        sl = slice(i * b_core, (i + 1) * b_core)
        yh = full["y_hists"][sl].astype(np.float32)      # [b_core, 12, 11]
        yt = full["y_targs"][sl].astype(np.float32)      # [b_core, 5, 11]
        m = {
            "input_encoded": np.ascontiguousarray(full["input_encoded"][sl]),
            "yhA_T": np.ascontiguousarray(yh[:, 0:6, :].reshape(b_core, 66).T),
            "yhB_T": np.ascontiguousarray(yh[:, 6:12, :].reshape(b_core, 66).T),
            "ytT_T": np.ascontiguousarray(yt.reshape(b_core, 55).T),
        }
        m.update(wk)
        in_maps.append(m)
    return in_maps


def kernel(**inputs) -> np.ndarray:
    full = {k: np.asarray(v, dtype=np.float32) for k, v in inputs.items()}
    b_core = full["input_encoded"].shape[0] // NCORES
    nc = build_program(b_core)
    in_maps = shard_inputs(full, b_core)
    res = bass_utils.run_bass_kernel_spmd(nc, in_maps, core_ids=list(range(NCORES)))
    out = np.concatenate([res.results[i]["out"] for i in range(NCORES)], axis=0)
    return out.astype(np.float32)
